# revision 8
# baseline (speedup 1.0000x reference)
"""Trainium2 Bass kernel for nn_MultiHeadAttention_59760174957330.

Shapes: B=2, S=2048, D=1024, H=16, d=64. The reference's
`k.reshape(B*H, S, d)` is a raw memory view (no head transpose), so each
"attention group" m = b*16 + m* is self-attention over a 128-token sequence
band (rows s in [128*m*, 128*(m*+1))) viewed as [2048, 64]: position
t = 16*s' + h carries head h's 64 features at band row s'.

Sharding: band-parallel — core c owns the 4 groups (b in {0,1}) x
(m* in {2c, 2c+1}). Projections are computed per-core for just that core's
512 sequence rows (full D), weights replicated. The output projection +
LayerNorm run as a second data-parallel NEFF over flattened rows after a
host reshard of the small (16 MB) context tensor.

Device pipeline per group (NEFF 1):
  S~^T tiles = k_chunk q^T (PE fp32r, head-major/t~ rows, t-ordered cols via
  strided APs), P^T = exp(0.5 S~^T) (ACT -> bf16),
  ctx = V~^T P^T accumulated in PSUM with a ones-column producing softmax
  row-sums for free; then the output pass A = exp(0.5 S - ln rowsum)
  fuses normalization into the ACT exp and streams straight to DRAM in
  natural (t1, t2) order. Context is normalized producer-side.
NEFF 2: out = context @ Wf + residual -> LayerNorm, 512 rows/core.
"""
import numpy as np

import concourse.bass as bass
import concourse.mybir as mybir
import concourse.tile as tile
from concourse import bacc
from concourse.bass_utils import run_bass_kernel_spmd
from concourse.masks import make_identity

F32 = mybir.dt.float32
F32R = mybir.dt.float32r
BF16 = mybir.dt.bfloat16
EXP = mybir.ActivationFunctionType.Exp
LN_ = mybir.ActivationFunctionType.Ln
SQRT = mybir.ActivationFunctionType.Sqrt

B, S, D, H = 2, 2048, 1024, 16
DH = D // H           # 64
NCORES = 8
BPC = 2               # bands (m*) per core
NG = B * BPC          # 4 groups per core
ROWS = NG * 128       # 512 seq rows per core
BS = B * S
RPC = BS // NCORES    # 512 flat rows per core in NEFF 2
LN_EPS = 1e-5
SCALE = float(DH // H) ** (-0.5)  # 0.5

T = S                 # positions per group (2048 = 128 s' x 16 h)
NKC = T // 128        # 16 t~ chunks (= heads)
NQG = T // 512        # 4
NQC = T // 128        # 16 t1 chunks
NK = D // 128         # 8 contraction chunks

_CACHE = {}


def _build_kernel1():
    nc = bacc.Bacc("TRN2", target_bir_lowering=False, debug=False, num_devices=NCORES)
    xq = nc.dram_tensor("xq", [D, ROWS], F32, kind="ExternalInput")
    xk = nc.dram_tensor("xk", [D, ROWS], F32, kind="ExternalInput")
    xv = nc.dram_tensor("xv", [D, ROWS], F32, kind="ExternalInput")
    wq = nc.dram_tensor("wq", [D, D], F32, kind="ExternalInput")
    wk = nc.dram_tensor("wk", [D, D], F32, kind="ExternalInput")
    wv = nc.dram_tensor("wv", [D, D], F32, kind="ExternalInput")
    bq = nc.dram_tensor("bq", [DH, H], F32, kind="ExternalInput")
    bk = nc.dram_tensor("bk", [DH, H], F32, kind="ExternalInput")
    bv = nc.dram_tensor("bv", [DH, H], F32, kind="ExternalInput")
    attn = nc.dram_tensor("attn", [NG, T, T], F32, kind="ExternalOutput")
    ctxT = nc.dram_tensor("ctxT", [D, ROWS], F32, kind="ExternalOutput")

    with tile.TileContext(nc) as tc:
        with (
            tc.tile_pool(name="consts", bufs=1) as consts,
            tc.tile_pool(name="pt", bufs=3) as ptp,
            tc.tile_pool(name="asb", bufs=2) as asbp,
            tc.tile_pool(name="small", bufs=2) as small,
            tc.tile_pool(name="dram", bufs=4, space="DRAM") as dramp,
            tc.tile_pool(name="ps", bufs=4, space="PSUM") as ps,
            tc.tile_pool(name="ps_acc", bufs=4, space="PSUM") as ps_acc,
        ):
            # t-ordered projected activations: [64, group, t = 16 s' + h]
            q_t = consts.tile([DH, NG, T], F32R)
            k_t = consts.tile([DH, NG, T], F32R)
            # V~ blocks per (group, head): [128 s', 65] with ones at col 64
            v_sb = consts.tile([128, NG, H, DH + 1], BF16)
            nc.vector.memset(v_sb[:, :, :, DH:DH + 1], 1.0)
            ident = consts.tile([DH, DH], F32)
            make_identity(nc, ident)

            # ---------- phase A: projections (M=64 per head) ----------
            with tc.tile_pool(name="pa", bufs=1) as pa:
                for tname, x_in, w_in, b_in in (
                    ("q", xq, wq, bq), ("k", xk, wk, bk), ("v", xv, wv, bv),
                ):
                    x_blk = pa.tile([128, NK, ROWS], F32R, tag="xblk", name=f"xb_{tname}", bufs=2)
                    nc.gpsimd.dma_start(
                        out=x_blk, in_=x_in.ap().rearrange("(kd p) r -> p kd r", p=128)
                    )
                    w_sb = pa.tile([128, NK, D], F32R, tag="wsb", name=f"w_{tname}", bufs=1)
                    nc.gpsimd.dma_start(
                        out=w_sb, in_=w_in.ap().rearrange("(kd p) n -> p kd n", p=128)
                    )
                    b_sb = pa.tile([DH, H], F32, tag="bsb", name=f"b_{tname}", bufs=3)
                    nc.gpsimd.dma_start(out=b_sb, in_=b_in.ap())

                    for h in range(H):
                        acc = ps.tile([DH, ROWS], F32, tag="ps", name=f"pj_{tname}{h}")
                        for kd in range(NK):
                            nc.tensor.matmul(
                                acc,
                                lhsT=w_sb[:, kd, h * DH:(h + 1) * DH],
                                rhs=x_blk[:, kd, :],
                                start=(kd == 0), stop=(kd == NK - 1),
                            )
                        if tname == "v":
                            vT_h = pa.tile([DH, NG, 128], F32, tag="vth", name=f"vth{h}", bufs=2)
                            nc.vector.tensor_scalar_add(
                                out=vT_h,
                                in0=acc.rearrange("p (g s) -> p g s", g=NG),
                                scalar1=b_sb[:, h:h + 1],
                            )
                            for g in range(NG):
                                tr_ps = ps.tile([128, DH], F32, tag="ps", name=f"tr{g}_{h}")
                                nc.tensor.transpose(tr_ps, vT_h[:, g, :], ident)
                                nc.vector.tensor_copy(v_sb[:, g, h, 0:DH], tr_ps)
                        else:
                            dest = {"q": q_t, "k": k_t}[tname]
                            dview = dest.rearrange("p g (s h) -> p g h s", h=H)
                            nc.vector.tensor_scalar_add(
                                out=dview[:, :, h, :],
                                in0=acc.rearrange("p (g s) -> p g s", g=NG),
                                scalar1=b_sb[:, h:h + 1],
                            )

            # head-major view: [p, g, h, s'] with s'-stride 16
            k_hm = k_t.rearrange("p g (s h) -> p g h s", h=H)

            # ---------- phase B: attention per group ----------
            for g in range(NG):
                # B1: S~^T -> exp -> P^T (bf16, rows t~/head-major, cols t1-order)
                ctx_ps = [
                    ps_acc.tile([DH + 1, 512], F32, tag="ctx", name=f"ctx{g}_{i}")
                    for i in range(NQG)
                ]
                for kc in range(NKC):
                    pt_t = ptp.tile([128, T], BF16, tag="pt", name=f"pt{g}_{kc}")
                    for qg in range(NQG):
                        st_ps = ps.tile([128, 512], F32, tag="ps", name=f"st{g}_{kc}_{qg}")
                        nc.tensor.matmul(
                            st_ps,
                            lhsT=k_hm[:, g, kc, :],
                            rhs=q_t[:, g, qg * 512:(qg + 1) * 512],
                            start=True, stop=True,
                        )
                        nc.scalar.activation(
                            out=pt_t[:, qg * 512:(qg + 1) * 512], in_=st_ps,
                            func=EXP, scale=SCALE,
                        )
                    for qg in range(NQG):
                        nc.tensor.matmul(
                            ctx_ps[qg],
                            lhsT=v_sb[:, g, kc, :],
                            rhs=pt_t[:, qg * 512:(qg + 1) * 512],
                            start=(kc == 0), stop=(kc == NKC - 1),
                        )

                # rowsum chain (all in t1-order)
                ctx_sb = small.tile([DH + 1, T], F32, tag="ctxsb", name=f"cs{g}", bufs=2)
                for qg in range(NQG):
                    nc.vector.tensor_copy(
                        ctx_sb[DH:DH + 1, qg * 512:(qg + 1) * 512],
                        ctx_ps[qg][DH:DH + 1, :],
                    )
                rs_dram = dramp.tile([T], F32, tag="rsd", name=f"rsd{g}")
                nc.gpsimd.dma_start(out=rs_dram, in_=ctx_sb[DH:DH + 1, :])
                rsT = small.tile([128, NQC], F32, tag="rsT", name=f"rsT{g}", bufs=4)
                nc.gpsimd.dma_start(out=rsT, in_=rs_dram.rearrange("(c p) -> p c", p=128))
                recipT = small.tile([128, NQC], F32, tag="recipT", name=f"rcT{g}", bufs=4)
                nc.vector.reciprocal(out=recipT, in_=rsT)
                negln = small.tile([128, NQC], F32, tag="negln", name=f"nl{g}", bufs=4)
                nc.scalar.activation(out=negln, in_=recipT, func=LN_)
                recip_dram = dramp.tile([T], F32, tag="rcd", name=f"rcd{g}")
                nc.gpsimd.dma_start(
                    out=recip_dram.rearrange("(c p) -> p c", p=128), in_=recipT
                )
                recip_bc = small.tile([128, T], F32, tag="rbc", name=f"rbc{g}", bufs=2)
                nc.gpsimd.dma_start(out=recip_bc, in_=recip_dram.partition_broadcast(128))

                # normalize context (rows = within-head feature i, cols = t1)
                for qg in range(NQG):
                    nc.vector.tensor_mul(
                        out=ctx_sb[0:DH, qg * 512:(qg + 1) * 512],
                        in0=ctx_ps[qg][0:DH, :],
                        in1=recip_bc[0:DH, qg * 512:(qg + 1) * 512],
                    )
                # scatter to ctxT rows h*64+i, cols = band columns of this group
                cs_v = ctx_sb[0:DH, :].rearrange("p (s h) -> p h s", h=H)
                for h in range(H):
                    nc.gpsimd.dma_start(
                        out=ctxT.ap()[h * DH:(h + 1) * DH, g * 128:(g + 1) * 128],
                        in_=cs_v[:, h, :],
                    )

                # B2: A = exp(0.5 S - ln rowsum), natural (t1, t2) order
                for qc in range(NQC):
                    a_sb = asbp.tile([128, T], F32, tag="a", name=f"a{g}_{qc}")
                    for kg in range(NQG):
                        s_ps = ps.tile([128, 512], F32, tag="ps", name=f"s{g}_{qc}_{kg}")
                        nc.tensor.matmul(
                            s_ps,
                            lhsT=q_t[:, g, qc * 128:(qc + 1) * 128],
                            rhs=k_t[:, g, kg * 512:(kg + 1) * 512],
                            start=True, stop=True,
                        )
                        nc.scalar.activation(
                            out=a_sb[:, kg * 512:(kg + 1) * 512], in_=s_ps,
                            func=EXP, scale=SCALE, bias=negln[:, qc:qc + 1],
                        )
                    nc.gpsimd.dma_start(
                        out=attn.ap()[g, qc * 128:(qc + 1) * 128, :], in_=a_sb
                    )

    nc.compile()
    return nc


def _build_kernel2():
    nc = bacc.Bacc("TRN2", target_bir_lowering=False, debug=False, num_devices=NCORES)
    ctxT_r = nc.dram_tensor("ctxT_r", [D, RPC], F32, kind="ExternalInput")
    wf = nc.dram_tensor("wf", [D, D], F32, kind="ExternalInput")
    resid = nc.dram_tensor("resid", [RPC, D], F32, kind="ExternalInput")
    ln_g = nc.dram_tensor("ln_g", [D], F32, kind="ExternalInput")
    ln_b = nc.dram_tensor("ln_b", [D], F32, kind="ExternalInput")
    normed = nc.dram_tensor("normed", [RPC, D], F32, kind="ExternalOutput")

    NRC = RPC // 128  # 4
    NOG = D // 512    # 2

    with tile.TileContext(nc) as tc:
        with (
            tc.tile_pool(name="consts", bufs=1) as consts,
            tc.tile_pool(name="work", bufs=2) as work,
            tc.tile_pool(name="stat", bufs=4) as stat,
            tc.tile_pool(name="ps", bufs=4, space="PSUM") as ps,
        ):
            ctx_t = consts.tile([128, NK, RPC], F32R)
            nc.gpsimd.dma_start(out=ctx_t, in_=ctxT_r.ap().rearrange("(k p) r -> p k r", p=128))
            wf_t = consts.tile([128, NK, D], F32R)
            nc.gpsimd.dma_start(out=wf_t, in_=wf.ap().rearrange("(k p) n -> p k n", p=128))
            res_t = consts.tile([128, NRC, D], F32)
            nc.gpsimd.dma_start(out=res_t, in_=resid.ap().rearrange("(rc p) n -> p rc n", p=128))
            g_bc = consts.tile([128, D], F32)
            nc.gpsimd.dma_start(out=g_bc, in_=ln_g.ap().partition_broadcast(128))
            b_bc = consts.tile([128, D], F32)
            nc.gpsimd.dma_start(out=b_bc, in_=ln_b.ap().partition_broadcast(128))
            eps_t = consts.tile([128, 1], F32)
            nc.vector.memset(eps_t, LN_EPS)

            out_view = normed.ap().rearrange("(rc p) n -> p rc n", p=128)
            for rc in range(NRC):
                x_sb = work.tile([128, D], F32, tag="x", name=f"x{rc}")
                for og in range(NOG):
                    acc = ps.tile([128, 512], F32, tag="ps", name=f"acc{rc}_{og}")
                    for k in range(NK):
                        nc.tensor.matmul(
                            acc,
                            lhsT=ctx_t[:, k, rc * 128:(rc + 1) * 128],
                            rhs=wf_t[:, k, og * 512:(og + 1) * 512],
                            start=(k == 0), stop=(k == NK - 1),
                        )
                    nc.vector.tensor_add(
                        out=x_sb[:, og * 512:(og + 1) * 512],
                        in0=acc, in1=res_t[:, rc, og * 512:(og + 1) * 512],
                    )
                stats = stat.tile([128, 2, 6], F32, tag="st", name=f"st{rc}")
                for sg in range(2):
                    nc.vector.bn_stats(
                        out=stats[:, sg, :], in_=x_sb[:, sg * 512:(sg + 1) * 512]
                    )
                mv = stat.tile([128, 2], F32, tag="mv", name=f"mv{rc}")
                nc.vector.bn_aggr(out=mv, in_=stats)
                sd = stat.tile([128, 1], F32, tag="sd", name=f"sd{rc}")
                nc.scalar.activation(out=sd, in_=mv[:, 1:2], func=SQRT, bias=eps_t)
                rstd = stat.tile([128, 1], F32, tag="rstd", name=f"rst{rc}")
                nc.vector.reciprocal(out=rstd, in_=sd)
                y_sb = work.tile([128, D], F32, tag="y", name=f"y{rc}")
                nc.vector.tensor_scalar(
                    out=y_sb, in0=x_sb, scalar1=mv[:, 0:1], scalar2=rstd,
                    op0=mybir.AluOpType.subtract, op1=mybir.AluOpType.mult,
                )
                nc.vector.tensor_mul(out=y_sb, in0=y_sb, in1=g_bc)
                nc.vector.tensor_add(out=y_sb, in0=y_sb, in1=b_bc)
                nc.gpsimd.dma_start(out=out_view[:, rc, :], in_=y_sb)

    nc.compile()
    return nc


def kernel(key, value, query, Wk, bk, Wv, bv, Wq, bq, Wf, bf, ln_g, ln_b):
    f32 = lambda a: np.ascontiguousarray(np.asarray(a, dtype=np.float32))
    key, value, query = f32(key), f32(value), f32(query)
    Wk, Wv, Wq, Wf = f32(Wk), f32(Wv), f32(Wq), f32(Wf)
    bk, bv, bq, bf, ln_g, ln_b = f32(bk), f32(bv), f32(bq), f32(bf), f32(ln_g), f32(ln_b)

    # transposed activations [D, B*S]; per-core band column slices
    qT = query.reshape(BS, D).T
    kT = key.reshape(BS, D).T
    vT = value.reshape(BS, D).T

    def band_cols(c):
        # group order g: (b=0,m*=2c), (b=0,2c+1), (b=1,2c), (b=1,2c+1)
        cols = []
        for g in range(NG):
            b, mstar = g // BPC, 2 * c + g % BPC
            s0 = b * S + mstar * 128
            cols.append(np.arange(s0, s0 + 128))
        return np.concatenate(cols)

    if "nc1" not in _CACHE:
        _CACHE["nc1"] = _build_kernel1()
    nc1 = _CACHE["nc1"]

    bq2 = np.ascontiguousarray(bq.reshape(H, DH).T)
    bk2 = np.ascontiguousarray(bk.reshape(H, DH).T)
    bv2 = np.ascontiguousarray(bv.reshape(H, DH).T)
    in_maps1 = []
    for c in range(NCORES):
        cols = band_cols(c)
        in_maps1.append({
            "xq": np.ascontiguousarray(qT[:, cols]),
            "xk": np.ascontiguousarray(kT[:, cols]),
            "xv": np.ascontiguousarray(vT[:, cols]),
            "wq": Wq, "wk": Wk, "wv": Wv,
            "bq": bq2, "bk": bk2, "bv": bv2,
        })
    res1 = run_bass_kernel_spmd(nc1, in_maps1, core_ids=list(range(NCORES)))
    _CACHE["res1"] = res1

    attention = np.empty((B * H, S, S), dtype=np.float32)
    ctxT_full = np.empty((D, BS), dtype=np.float32)
    for c in range(NCORES):
        r = res1.results[c]
        cols = band_cols(c)
        for g in range(NG):
            b, mstar = g // BPC, 2 * c + g % BPC
            attention[b * H + mstar] = r["attn"][g]
        ctxT_full[:, cols] = r["ctxT"]

    if "nc2" not in _CACHE:
        _CACHE["nc2"] = _build_kernel2()
    nc2 = _CACHE["nc2"]

    query_flat = query.reshape(BS, D)
    in_maps2 = []
    for r in range(NCORES):
        rows = slice(r * RPC, (r + 1) * RPC)
        in_maps2.append({
            "ctxT_r": np.ascontiguousarray(ctxT_full[:, rows]),
            "wf": Wf,
            "resid": np.ascontiguousarray(query_flat[rows, :] + bf[None, :]),
            "ln_g": ln_g, "ln_b": ln_b,
        })
    res2 = run_bass_kernel_spmd(nc2, in_maps2, core_ids=list(range(NCORES)))
    _CACHE["res2"] = res2

    normed = np.empty((BS, D), dtype=np.float32)
    for r in range(NCORES):
        normed[r * RPC:(r + 1) * RPC, :] = res2.results[r]["normed"]
    normed = normed.reshape(B, S, D)

    return (normed, attention)


# revision 12
# speedup vs baseline: 12.9868x; 12.9868x over previous
"""Trainium2 Bass kernel for nn_MultiHeadAttention_59760174957330.

Shapes: B=2, S=2048, D=1024, H=16, d=64. The reference's
`k.reshape(B*H, S, d)` is a raw memory view (no head transpose), so each
"attention group" m = b*16 + m* is self-attention over a 128-token sequence
band (rows s in [128*m*, 128*(m*+1))) viewed as [2048, 64]: position
t = 16*s' + h carries head h's 64 features at band row s'.

Sharding: band-parallel — core c owns the 4 groups (b in {0,1}) x
(m* in {2c, 2c+1}). Projections are computed per-core for just that core's
512 sequence rows (full D), weights replicated. The output projection +
LayerNorm run as a second data-parallel NEFF over flattened rows after a
host reshard of the small (16 MB) context tensor.

Device pipeline per group (NEFF 1):
  S~^T tiles = k_chunk q^T (PE fp32r, head-major/t~ rows, t-ordered cols via
  strided APs), P^T = exp(0.5 S~^T) (ACT -> bf16),
  ctx = V~^T P^T accumulated in PSUM with a ones-column producing softmax
  row-sums for free; then the output pass A = exp(0.5 S - ln rowsum)
  fuses normalization into the ACT exp and streams straight to DRAM in
  natural (t1, t2) order. Context is normalized producer-side.
NEFF 2: out = context @ Wf + residual -> LayerNorm, 512 rows/core.
"""
import numpy as np

import concourse.bass as bass
import concourse.mybir as mybir
import concourse.tile as tile
from concourse import bacc
from concourse.bass_utils import run_bass_kernel_spmd
from concourse.masks import make_identity

F32 = mybir.dt.float32
F32R = mybir.dt.float32r
BF16 = mybir.dt.bfloat16
EXP = mybir.ActivationFunctionType.Exp
LN_ = mybir.ActivationFunctionType.Ln
SQRT = mybir.ActivationFunctionType.Sqrt

B, S, D, H = 2, 2048, 1024, 16
DH = D // H           # 64
NCORES = 8
BPC = 2               # bands (m*) per core
NG = B * BPC          # 4 groups per core
ROWS = NG * 128       # 512 seq rows per core
BS = B * S
RPC = BS // NCORES    # 512 flat rows per core in NEFF 2
LN_EPS = 1e-5
SCALE = float(DH // H) ** (-0.5)  # 0.5

T = S                 # positions per group (2048 = 128 s' x 16 h)
NKC = T // 128        # 16 t~ chunks (= heads)
NQG = T // 512        # 4
NQC = T // 128        # 16 t1 chunks
NK = D // 128         # 8 contraction chunks

_CACHE = {}


def _build_kernel1():
    nc = bacc.Bacc("TRN2", target_bir_lowering=False, debug=False, num_devices=NCORES)
    xq = nc.dram_tensor("xq", [D, ROWS], F32R, kind="ExternalInput")
    xk = nc.dram_tensor("xk", [D, ROWS], F32R, kind="ExternalInput")
    xv = nc.dram_tensor("xv", [D, ROWS], F32R, kind="ExternalInput")
    wq = nc.dram_tensor("wq", [D, D], F32R, kind="ExternalInput")
    wk = nc.dram_tensor("wk", [D, D], F32R, kind="ExternalInput")
    wv = nc.dram_tensor("wv", [D, D], F32R, kind="ExternalInput")
    bq = nc.dram_tensor("bq", [DH, H], F32, kind="ExternalInput")
    bk = nc.dram_tensor("bk", [DH, H], F32, kind="ExternalInput")
    bv = nc.dram_tensor("bv", [DH, H], F32, kind="ExternalInput")
    attn = nc.dram_tensor("attn", [NG, T, T], F32, kind="ExternalOutput")
    ctx_out = nc.dram_tensor("ctx_out", [NG, DH, T], F32, kind="ExternalOutput")
    rs_out = nc.dram_tensor("rs_out", [NG, T], F32, kind="ExternalOutput")

    with tile.TileContext(nc) as tc:
        with (
            tc.tile_pool(name="consts", bufs=1) as consts,
            tc.tile_pool(name="pt", bufs=3) as ptp,
            tc.tile_pool(name="asb", bufs=3) as asbp,
            tc.tile_pool(name="small", bufs=2) as small,
            tc.tile_pool(name="dram", bufs=4, space="DRAM") as dramp,
            tc.tile_pool(name="ps", bufs=4, space="PSUM") as ps,
            tc.tile_pool(name="ps_acc", bufs=4, space="PSUM") as ps_acc,
        ):
            # t-ordered projected activations: [64, group, t = 16 s' + h]
            q_t = consts.tile([DH, NG, T], F32R)
            k_t = consts.tile([DH, NG, T], F32R)
            # V~ blocks per (group, head): [128 s', 65] with ones at col 64
            v_sb = consts.tile([128, NG, H, DH + 1], BF16)
            nc.vector.memset(v_sb[:, :, :, DH:DH + 1], 1.0)
            ident = consts.tile([DH, DH], F32)
            make_identity(nc, ident)
            ones65 = consts.tile([DH + 1, 1], F32)
            nc.vector.memset(ones65, 1.0)

            # ---------- phase A: projections (M=64 per head) ----------
            with tc.tile_pool(name="pa", bufs=1) as pa:
                for tname, x_in, w_in, b_in in (
                    ("q", xq, wq, bq), ("k", xk, wk, bk), ("v", xv, wv, bv),
                ):
                    x_blk = pa.tile([128, NK, ROWS], F32R, tag="xblk", name=f"xb_{tname}", bufs=2)
                    nc.sync.dma_start(
                        out=x_blk, in_=x_in.ap().rearrange("(kd p) r -> p kd r", p=128)
                    )
                    w_sb = pa.tile([128, NK, D], F32R, tag="wsb", name=f"w_{tname}", bufs=1)
                    nc.sync.dma_start(
                        out=w_sb, in_=w_in.ap().rearrange("(kd p) n -> p kd n", p=128)
                    )
                    b_sb = pa.tile([DH, H], F32, tag="bsb", name=f"b_{tname}", bufs=3)
                    nc.gpsimd.dma_start(out=b_sb, in_=b_in.ap())

                    for h in range(H):
                        acc = ps.tile([DH, ROWS], F32, tag="ps", name=f"pj_{tname}{h}")
                        for kd in range(NK):
                            nc.tensor.matmul(
                                acc,
                                lhsT=w_sb[:, kd, h * DH:(h + 1) * DH],
                                rhs=x_blk[:, kd, :],
                                start=(kd == 0), stop=(kd == NK - 1),
                            )
                        if tname == "v":
                            vT_h = pa.tile([DH, NG, 128], F32, tag="vth", name=f"vth{h}", bufs=2)
                            nc.vector.tensor_scalar_add(
                                out=vT_h,
                                in0=acc.rearrange("p (g s) -> p g s", g=NG),
                                scalar1=b_sb[:, h:h + 1],
                            )
                            for g in range(NG):
                                tr_ps = ps.tile([128, DH], F32, tag="ps", name=f"tr{g}_{h}")
                                nc.tensor.transpose(tr_ps, vT_h[:, g, :], ident)
                                nc.vector.tensor_copy(v_sb[:, g, h, 0:DH], tr_ps)
                        else:
                            dest = {"q": q_t, "k": k_t}[tname]
                            dview = dest.rearrange("p g (s h) -> p g h s", h=H)
                            nc.vector.tensor_scalar_add(
                                out=dview[:, :, h, :],
                                in0=acc.rearrange("p (g s) -> p g s", g=NG),
                                scalar1=b_sb[:, h:h + 1],
                            )

            # head-major view: [p, g, h, s'] with s'-stride 16
            k_hm = k_t.rearrange("p g (s h) -> p g h s", h=H)

            # ---------- phase B: attention per group ----------
            for g in range(NG):
                # B1: S~^T -> exp -> P^T (bf16, rows t~/head-major, cols t1-order)
                ctx_ps = [
                    ps_acc.tile([DH + 1, 512], F32, tag="ctx", name=f"ctx{g}_{i}")
                    for i in range(NQG)
                ]
                for kc in range(NKC):
                    pt_t = ptp.tile([128, T], BF16, tag="pt", name=f"pt{g}_{kc}")
                    for qg in range(NQG):
                        st_ps = ps.tile([128, 512], F32, tag="ps", name=f"st{g}_{kc}_{qg}")
                        nc.tensor.matmul(
                            st_ps,
                            lhsT=k_hm[:, g, kc, :],
                            rhs=q_t[:, g, qg * 512:(qg + 1) * 512],
                            start=True, stop=True,
                        )
                        nc.scalar.activation(
                            out=pt_t[:, qg * 512:(qg + 1) * 512], in_=st_ps,
                            func=EXP, scale=SCALE,
                        )
                    for qg in range(NQG):
                        nc.tensor.matmul(
                            ctx_ps[qg],
                            lhsT=v_sb[:, g, kc, :],
                            rhs=pt_t[:, qg * 512:(qg + 1) * 512],
                            start=(kc == 0), stop=(kc == NKC - 1),
                        )

                # evacuate unnormalized context + rowsum row, contiguous outputs
                ctx_sb = small.tile([DH + 1, T], F32, tag="ctxsb", name=f"cs{g}", bufs=2)
                for qg in range(NQG):
                    nc.vector.tensor_copy(
                        ctx_sb[:, qg * 512:(qg + 1) * 512], ctx_ps[qg]
                    )
                nc.sync.dma_start(out=ctx_out.ap()[g, :, :], in_=ctx_sb[0:DH, :])
                nc.sync.dma_start(out=rs_out.ap()[g, :], in_=ctx_sb[DH:DH + 1, :])

                # rowsum -> [128 t1, 16 chunk] via 16 K=1 matmuls (PE transpose)
                rsT_ps = ps.tile([128, NQC], F32, tag="ps", name=f"rsT{g}")
                for c16 in range(NQC):
                    nc.tensor.matmul(
                        rsT_ps[:, c16:c16 + 1],
                        lhsT=ctx_sb[DH:DH + 1, c16 * 128:(c16 + 1) * 128],
                        rhs=ones65[DH:DH + 1, :],
                        start=True, stop=True,
                    )
                recipT = small.tile([128, NQC], F32, tag="recipT", name=f"rcT{g}", bufs=4)
                nc.vector.reciprocal(out=recipT, in_=rsT_ps)
                negln = small.tile([128, NQC], F32, tag="negln", name=f"nl{g}", bufs=4)
                nc.scalar.activation(out=negln, in_=recipT, func=LN_)

                # B2: A = exp(0.5 S - ln rowsum), natural (t1, t2) order
                for qc in range(NQC):
                    a_sb = asbp.tile([128, T], F32, tag="a", name=f"a{g}_{qc}")
                    for kg in range(NQG):
                        s_ps = ps.tile([128, 512], F32, tag="ps", name=f"s{g}_{qc}_{kg}")
                        nc.tensor.matmul(
                            s_ps,
                            lhsT=q_t[:, g, qc * 128:(qc + 1) * 128],
                            rhs=k_t[:, g, kg * 512:(kg + 1) * 512],
                            start=True, stop=True,
                        )
                        nc.scalar.activation(
                            out=a_sb[:, kg * 512:(kg + 1) * 512], in_=s_ps,
                            func=EXP, scale=SCALE, bias=negln[:, qc:qc + 1],
                        )
                    nc.sync.dma_start(
                        out=attn.ap()[g, qc * 128:(qc + 1) * 128, :], in_=a_sb
                    )

    nc.compile()
    return nc


def _build_kernel2():
    nc = bacc.Bacc("TRN2", target_bir_lowering=False, debug=False, num_devices=NCORES)
    ctxT_r = nc.dram_tensor("ctxT_r", [D, RPC], F32, kind="ExternalInput")
    scale_r = nc.dram_tensor("scale_r", [D, RPC], F32, kind="ExternalInput")
    wf = nc.dram_tensor("wf", [D, D], F32, kind="ExternalInput")
    resid = nc.dram_tensor("resid", [RPC, D], F32, kind="ExternalInput")
    ln_g = nc.dram_tensor("ln_g", [D], F32, kind="ExternalInput")
    ln_b = nc.dram_tensor("ln_b", [D], F32, kind="ExternalInput")
    normed = nc.dram_tensor("normed", [RPC, D], F32, kind="ExternalOutput")

    NRC = RPC // 128  # 4
    NOG = D // 512    # 2

    with tile.TileContext(nc) as tc:
        with (
            tc.tile_pool(name="consts", bufs=1) as consts,
            tc.tile_pool(name="work", bufs=2) as work,
            tc.tile_pool(name="stat", bufs=4) as stat,
            tc.tile_pool(name="ps", bufs=4, space="PSUM") as ps,
        ):
            ctx_raw = consts.tile([128, NK, RPC], F32)
            nc.sync.dma_start(out=ctx_raw, in_=ctxT_r.ap().rearrange("(k p) r -> p k r", p=128))
            sc_t = consts.tile([128, NK, RPC], F32)
            nc.sync.dma_start(out=sc_t, in_=scale_r.ap().rearrange("(k p) r -> p k r", p=128))
            ctx_t = consts.tile([128, NK, RPC], F32R)
            nc.vector.tensor_mul(out=ctx_t, in0=ctx_raw, in1=sc_t)
            wf_t = consts.tile([128, NK, D], F32R)
            nc.gpsimd.dma_start(out=wf_t, in_=wf.ap().rearrange("(k p) n -> p k n", p=128))
            res_t = consts.tile([128, NRC, D], F32)
            nc.gpsimd.dma_start(out=res_t, in_=resid.ap().rearrange("(rc p) n -> p rc n", p=128))
            g_bc = consts.tile([128, D], F32)
            nc.gpsimd.dma_start(out=g_bc, in_=ln_g.ap().partition_broadcast(128))
            b_bc = consts.tile([128, D], F32)
            nc.gpsimd.dma_start(out=b_bc, in_=ln_b.ap().partition_broadcast(128))
            eps_t = consts.tile([128, 1], F32)
            nc.vector.memset(eps_t, LN_EPS)

            out_view = normed.ap().rearrange("(rc p) n -> p rc n", p=128)
            for rc in range(NRC):
                x_sb = work.tile([128, D], F32, tag="x", name=f"x{rc}")
                for og in range(NOG):
                    acc = ps.tile([128, 512], F32, tag="ps", name=f"acc{rc}_{og}")
                    for k in range(NK):
                        nc.tensor.matmul(
                            acc,
                            lhsT=ctx_t[:, k, rc * 128:(rc + 1) * 128],
                            rhs=wf_t[:, k, og * 512:(og + 1) * 512],
                            start=(k == 0), stop=(k == NK - 1),
                        )
                    nc.vector.tensor_add(
                        out=x_sb[:, og * 512:(og + 1) * 512],
                        in0=acc, in1=res_t[:, rc, og * 512:(og + 1) * 512],
                    )
                stats = stat.tile([128, 2, 6], F32, tag="st", name=f"st{rc}")
                for sg in range(2):
                    nc.vector.bn_stats(
                        out=stats[:, sg, :], in_=x_sb[:, sg * 512:(sg + 1) * 512]
                    )
                mv = stat.tile([128, 2], F32, tag="mv", name=f"mv{rc}")
                nc.vector.bn_aggr(out=mv, in_=stats)
                sd = stat.tile([128, 1], F32, tag="sd", name=f"sd{rc}")
                nc.scalar.activation(out=sd, in_=mv[:, 1:2], func=SQRT, bias=eps_t)
                rstd = stat.tile([128, 1], F32, tag="rstd", name=f"rst{rc}")
                nc.vector.reciprocal(out=rstd, in_=sd)
                y_sb = work.tile([128, D], F32, tag="y", name=f"y{rc}")
                nc.vector.tensor_scalar(
                    out=y_sb, in0=x_sb, scalar1=mv[:, 0:1], scalar2=rstd,
                    op0=mybir.AluOpType.subtract, op1=mybir.AluOpType.mult,
                )
                nc.vector.tensor_mul(out=y_sb, in0=y_sb, in1=g_bc)
                nc.vector.tensor_add(out=y_sb, in0=y_sb, in1=b_bc)
                nc.gpsimd.dma_start(out=out_view[:, rc, :], in_=y_sb)

    nc.compile()
    return nc


def kernel(key, value, query, Wk, bk, Wv, bv, Wq, bq, Wf, bf, ln_g, ln_b):
    f32 = lambda a: np.ascontiguousarray(np.asarray(a, dtype=np.float32))
    key, value, query = f32(key), f32(value), f32(query)
    Wk, Wv, Wq, Wf = f32(Wk), f32(Wv), f32(Wq), f32(Wf)
    bk, bv, bq, bf, ln_g, ln_b = f32(bk), f32(bv), f32(bq), f32(bf), f32(ln_g), f32(ln_b)

    # transposed activations [D, B*S]; per-core band column slices
    qT = query.reshape(BS, D).T
    kT = key.reshape(BS, D).T
    vT = value.reshape(BS, D).T

    def band_cols(c):
        # group order g: (b=0,m*=2c), (b=0,2c+1), (b=1,2c), (b=1,2c+1)
        cols = []
        for g in range(NG):
            b, mstar = g // BPC, 2 * c + g % BPC
            s0 = b * S + mstar * 128
            cols.append(np.arange(s0, s0 + 128))
        return np.concatenate(cols)

    if "nc1" not in _CACHE:
        _CACHE["nc1"] = _build_kernel1()
    nc1 = _CACHE["nc1"]

    bq2 = np.ascontiguousarray(bq.reshape(H, DH).T)
    bk2 = np.ascontiguousarray(bk.reshape(H, DH).T)
    bv2 = np.ascontiguousarray(bv.reshape(H, DH).T)
    in_maps1 = []
    for c in range(NCORES):
        cols = band_cols(c)
        in_maps1.append({
            "xq": np.ascontiguousarray(qT[:, cols]),
            "xk": np.ascontiguousarray(kT[:, cols]),
            "xv": np.ascontiguousarray(vT[:, cols]),
            "wq": Wq, "wk": Wk, "wv": Wv,
            "bq": bq2, "bk": bk2, "bv": bv2,
        })
    res1 = run_bass_kernel_spmd(nc1, in_maps1, core_ids=list(range(NCORES)))
    _CACHE["res1"] = res1

    attention = np.empty((B * H, S, S), dtype=np.float32)
    ctxT_full = np.empty((D, BS), dtype=np.float32)
    scale_full = np.empty((D, BS), dtype=np.float32)
    for c in range(NCORES):
        r = res1.results[c]
        for g in range(NG):
            b, mstar = g // BPC, 2 * c + g % BPC
            attention[b * H + mstar] = r["attn"][g]
            fr0 = b * S + mstar * 128
            cg = r["ctx_out"][g].reshape(DH, 128, H)         # (i, s', h)
            ctxT_full[:, fr0:fr0 + 128] = (
                cg.transpose(2, 0, 1).reshape(D, 128)        # rows h*64+i
            )
            rsg = r["rs_out"][g].reshape(128, H)             # (s', h)
            scale_full[:, fr0:fr0 + 128] = np.repeat(
                (1.0 / rsg).T, DH, axis=0                    # rows h*64+i
            )

    if "nc2" not in _CACHE:
        _CACHE["nc2"] = _build_kernel2()
    nc2 = _CACHE["nc2"]

    query_flat = query.reshape(BS, D)
    in_maps2 = []
    for r in range(NCORES):
        rows = slice(r * RPC, (r + 1) * RPC)
        in_maps2.append({
            "ctxT_r": np.ascontiguousarray(ctxT_full[:, rows]),
            "scale_r": np.ascontiguousarray(scale_full[:, rows]),
            "wf": Wf,
            "resid": np.ascontiguousarray(query_flat[rows, :] + bf[None, :]),
            "ln_g": ln_g, "ln_b": ln_b,
        })
    res2 = run_bass_kernel_spmd(nc2, in_maps2, core_ids=list(range(NCORES)))
    _CACHE["res2"] = res2

    normed = np.empty((BS, D), dtype=np.float32)
    for r in range(NCORES):
        normed[r * RPC:(r + 1) * RPC, :] = res2.results[r]["normed"]
    normed = normed.reshape(B, S, D)

    return (normed, attention)


# revision 14
# speedup vs baseline: 13.4028x; 1.0320x over previous
"""Trainium2 Bass kernel for nn_MultiHeadAttention_59760174957330.

Shapes: B=2, S=2048, D=1024, H=16, d=64. The reference's
`k.reshape(B*H, S, d)` is a raw memory view (no head transpose), so each
"attention group" m = b*16 + m* is self-attention over a 128-token sequence
band (rows s in [128*m*, 128*(m*+1))) viewed as [2048, 64]: position
t = 16*s' + h carries head h's 64 features at band row s'.

Sharding: band-parallel — core c owns the 4 groups (b in {0,1}) x
(m* in {2c, 2c+1}). Projections are computed per-core for just that core's
512 sequence rows (full D), weights replicated. The output projection +
LayerNorm run as a second data-parallel NEFF over flattened rows after a
host reshard of the small (16 MB) context tensor.

Device pipeline per group (NEFF 1):
  S~^T tiles = k_chunk q^T (PE fp32r, head-major/t~ rows, t-ordered cols via
  strided APs), P^T = exp(0.5 S~^T) (ACT -> bf16),
  ctx = V~^T P^T accumulated in PSUM with a ones-column producing softmax
  row-sums for free; then the output pass A = exp(0.5 S - ln rowsum)
  fuses normalization into the ACT exp and streams straight to DRAM in
  natural (t1, t2) order. Context is normalized producer-side.
NEFF 2: out = context @ Wf + residual -> LayerNorm, 512 rows/core.
"""
import numpy as np

import concourse.bass as bass
import concourse.mybir as mybir
import concourse.tile as tile
from concourse import bacc
from concourse.bass_utils import run_bass_kernel_spmd
from concourse.masks import make_identity

F32 = mybir.dt.float32
F32R = mybir.dt.float32r
BF16 = mybir.dt.bfloat16
FP16 = mybir.dt.float16
EXP = mybir.ActivationFunctionType.Exp
LN_ = mybir.ActivationFunctionType.Ln
SQRT = mybir.ActivationFunctionType.Sqrt

B, S, D, H = 2, 2048, 1024, 16
DH = D // H           # 64
NCORES = 8
BPC = 2               # bands (m*) per core
NG = B * BPC          # 4 groups per core
ROWS = NG * 128       # 512 seq rows per core
BS = B * S
RPC = BS // NCORES    # 512 flat rows per core in NEFF 2
LN_EPS = 1e-5
SCALE = float(DH // H) ** (-0.5)  # 0.5

T = S                 # positions per group (2048 = 128 s' x 16 h)
NKC = T // 128        # 16 t~ chunks (= heads)
NQG = T // 512        # 4
NQC = T // 128        # 16 t1 chunks
NK = D // 128         # 8 contraction chunks

_CACHE = {}


def _build_kernel1():
    nc = bacc.Bacc("TRN2", target_bir_lowering=False, debug=False, num_devices=NCORES)
    xq = nc.dram_tensor("xq", [D, ROWS], F32R, kind="ExternalInput")
    xk = nc.dram_tensor("xk", [D, ROWS], F32R, kind="ExternalInput")
    xv = nc.dram_tensor("xv", [D, ROWS], F32R, kind="ExternalInput")
    wq = nc.dram_tensor("wq", [D, D], F32R, kind="ExternalInput")
    wk = nc.dram_tensor("wk", [D, D], F32R, kind="ExternalInput")
    wv = nc.dram_tensor("wv", [D, D], F32R, kind="ExternalInput")
    bq = nc.dram_tensor("bq", [DH, H], F32, kind="ExternalInput")
    bk = nc.dram_tensor("bk", [DH, H], F32, kind="ExternalInput")
    bv = nc.dram_tensor("bv", [DH, H], F32, kind="ExternalInput")
    attn = nc.dram_tensor("attn", [NG, T, T], F32, kind="ExternalOutput")
    ctx_out = nc.dram_tensor("ctx_out", [NG, DH, T], F32, kind="ExternalOutput")
    rs_out = nc.dram_tensor("rs_out", [NG, T], F32, kind="ExternalOutput")

    with tile.TileContext(nc) as tc:
        with (
            tc.tile_pool(name="consts", bufs=1) as consts,
            tc.tile_pool(name="pt", bufs=3) as ptp,
            tc.tile_pool(name="asb", bufs=3) as asbp,
            tc.tile_pool(name="small", bufs=2) as small,
            tc.tile_pool(name="dram", bufs=4, space="DRAM") as dramp,
            tc.tile_pool(name="ps", bufs=2, space="PSUM") as ps,
            tc.tile_pool(name="ps_acc", bufs=4, space="PSUM") as ps_acc,
        ):
            # t-ordered projected activations: [64, group, t = 16 s' + h]
            q_t = consts.tile([DH, NG, T], F32R)
            k_t = consts.tile([DH, NG, T], F32R)
            # V~ blocks per (group, head): [128 s', 65] with ones at col 64
            v_sb = consts.tile([128, NG, H, DH + 1], BF16)
            nc.vector.memset(v_sb[:, :, :, DH:DH + 1], 1.0)
            ident = consts.tile([DH, DH], F32)
            make_identity(nc, ident)
            ones65 = consts.tile([DH + 1, 1], F32)
            nc.vector.memset(ones65, 1.0)

            # ---------- phase A: projections (M=64 per head) ----------
            with tc.tile_pool(name="pa", bufs=1) as pa:
                for tname, x_in, w_in, b_in in (
                    ("q", xq, wq, bq), ("k", xk, wk, bk), ("v", xv, wv, bv),
                ):
                    x_blk = pa.tile([128, NK, ROWS], F32R, tag="xblk", name=f"xb_{tname}", bufs=2)
                    nc.sync.dma_start(
                        out=x_blk, in_=x_in.ap().rearrange("(kd p) r -> p kd r", p=128)
                    )
                    w_sb = pa.tile([128, NK, D], F32R, tag="wsb", name=f"w_{tname}", bufs=1)
                    nc.sync.dma_start(
                        out=w_sb, in_=w_in.ap().rearrange("(kd p) n -> p kd n", p=128)
                    )
                    b_sb = pa.tile([DH, H], F32, tag="bsb", name=f"b_{tname}", bufs=3)
                    nc.gpsimd.dma_start(out=b_sb, in_=b_in.ap())

                    for h in range(H):
                        acc = ps.tile([DH, ROWS], F32, tag="ps", name=f"pj_{tname}{h}")
                        for kd in range(NK):
                            nc.tensor.matmul(
                                acc,
                                lhsT=w_sb[:, kd, h * DH:(h + 1) * DH],
                                rhs=x_blk[:, kd, :],
                                start=(kd == 0), stop=(kd == NK - 1),
                            )
                        if tname == "v":
                            vT_h = pa.tile([DH, NG, 128], F32, tag="vth", name=f"vth{h}", bufs=2)
                            nc.vector.tensor_scalar_add(
                                out=vT_h,
                                in0=acc.rearrange("p (g s) -> p g s", g=NG),
                                scalar1=b_sb[:, h:h + 1],
                            )
                            for g in range(NG):
                                tr_ps = ps.tile([128, DH], F32, tag="ps", name=f"tr{g}_{h}")
                                nc.tensor.transpose(tr_ps, vT_h[:, g, :], ident)
                                nc.vector.tensor_copy(v_sb[:, g, h, 0:DH], tr_ps)
                        else:
                            dest = {"q": q_t, "k": k_t}[tname]
                            dview = dest.rearrange("p g (s h) -> p g h s", h=H)
                            nc.vector.tensor_scalar_add(
                                out=dview[:, :, h, :],
                                in0=acc.rearrange("p (g s) -> p g s", g=NG),
                                scalar1=b_sb[:, h:h + 1],
                            )

            # head-major view: [p, g, h, s'] with s'-stride 16
            k_hm = k_t.rearrange("p g (s h) -> p g h s", h=H)

            # ---------- phase B: attention per group ----------
            for g in range(NG):
                # B1: S~^T -> exp -> P^T (bf16, rows t~/head-major, cols t1-order)
                ctx_ps = [
                    ps_acc.tile([DH + 1, 512], F32, tag="ctx", name=f"ctx{g}_{i}")
                    for i in range(NQG)
                ]
                for kc in range(NKC):
                    pt_t = ptp.tile([128, T], BF16, tag="pt", name=f"pt{g}_{kc}")
                    for half in range(2):
                        st_ps = ps.tile([128, 1024], F32, tag="ps", name=f"st{g}_{kc}_{half}")
                        for sub in range(2):
                            qg = half * 2 + sub
                            nc.tensor.matmul(
                                st_ps[:, sub * 512:(sub + 1) * 512],
                                lhsT=k_hm[:, g, kc, :],
                                rhs=q_t[:, g, qg * 512:(qg + 1) * 512],
                                start=True, stop=True,
                            )
                        nc.scalar.activation(
                            out=pt_t[:, half * 1024:(half + 1) * 1024], in_=st_ps,
                            func=EXP, scale=SCALE,
                        )
                    for qg in range(NQG):
                        nc.tensor.matmul(
                            ctx_ps[qg],
                            lhsT=v_sb[:, g, kc, :],
                            rhs=pt_t[:, qg * 512:(qg + 1) * 512],
                            start=(kc == 0), stop=(kc == NKC - 1),
                        )

                # evacuate unnormalized context + rowsum row, contiguous outputs
                ctx_sb = small.tile([DH + 1, T], F32, tag="ctxsb", name=f"cs{g}", bufs=2)
                for qg in range(NQG):
                    nc.vector.tensor_copy(
                        ctx_sb[:, qg * 512:(qg + 1) * 512], ctx_ps[qg]
                    )
                nc.sync.dma_start(out=ctx_out.ap()[g, :, :], in_=ctx_sb[0:DH, :])
                nc.sync.dma_start(out=rs_out.ap()[g, :], in_=ctx_sb[DH:DH + 1, :])

                # rowsum -> [128 t1, 16 chunk] via 16 K=1 matmuls (PE transpose)
                rsT_ps = ps.tile([128, NQC], F32, tag="ps", name=f"rsT{g}")
                for c16 in range(NQC):
                    nc.tensor.matmul(
                        rsT_ps[:, c16:c16 + 1],
                        lhsT=ctx_sb[DH:DH + 1, c16 * 128:(c16 + 1) * 128],
                        rhs=ones65[DH:DH + 1, :],
                        start=True, stop=True,
                    )
                recipT = small.tile([128, NQC], F32, tag="recipT", name=f"rcT{g}", bufs=4)
                nc.vector.reciprocal(out=recipT, in_=rsT_ps)
                negln = small.tile([128, NQC], F32, tag="negln", name=f"nl{g}", bufs=4)
                nc.scalar.activation(out=negln, in_=recipT, func=LN_)

                # B2: A = exp(0.5 S - ln rowsum), natural (t1, t2) order
                for qc in range(NQC):
                    a_sb = asbp.tile([128, T], F32, tag="a", name=f"a{g}_{qc}")
                    for half in range(2):
                        s_ps = ps.tile([128, 1024], F32, tag="ps", name=f"s{g}_{qc}_{half}")
                        for sub in range(2):
                            kg = half * 2 + sub
                            nc.tensor.matmul(
                                s_ps[:, sub * 512:(sub + 1) * 512],
                                lhsT=q_t[:, g, qc * 128:(qc + 1) * 128],
                                rhs=k_t[:, g, kg * 512:(kg + 1) * 512],
                                start=True, stop=True,
                            )
                        nc.scalar.activation(
                            out=a_sb[:, half * 1024:(half + 1) * 1024], in_=s_ps,
                            func=EXP, scale=SCALE, bias=negln[:, qc:qc + 1],
                        )
                    nc.sync.dma_start(
                        out=attn.ap()[g, qc * 128:(qc + 1) * 128, :], in_=a_sb
                    )

    nc.compile()
    return nc


def _build_kernel2():
    nc = bacc.Bacc("TRN2", target_bir_lowering=False, debug=False, num_devices=NCORES)
    ctxT_r = nc.dram_tensor("ctxT_r", [D, RPC], F32, kind="ExternalInput")
    scale_r = nc.dram_tensor("scale_r", [D, RPC], F32, kind="ExternalInput")
    wf = nc.dram_tensor("wf", [D, D], F32, kind="ExternalInput")
    resid = nc.dram_tensor("resid", [RPC, D], F32, kind="ExternalInput")
    ln_g = nc.dram_tensor("ln_g", [D], F32, kind="ExternalInput")
    ln_b = nc.dram_tensor("ln_b", [D], F32, kind="ExternalInput")
    normed = nc.dram_tensor("normed", [RPC, D], F32, kind="ExternalOutput")

    NRC = RPC // 128  # 4
    NOG = D // 512    # 2

    with tile.TileContext(nc) as tc:
        with (
            tc.tile_pool(name="consts", bufs=1) as consts,
            tc.tile_pool(name="work", bufs=2) as work,
            tc.tile_pool(name="stat", bufs=4) as stat,
            tc.tile_pool(name="ps", bufs=2, space="PSUM") as ps,
        ):
            ctx_raw = consts.tile([128, NK, RPC], F32)
            nc.sync.dma_start(out=ctx_raw, in_=ctxT_r.ap().rearrange("(k p) r -> p k r", p=128))
            sc_t = consts.tile([128, NK, RPC], F32)
            nc.sync.dma_start(out=sc_t, in_=scale_r.ap().rearrange("(k p) r -> p k r", p=128))
            ctx_t = consts.tile([128, NK, RPC], F32R)
            nc.vector.tensor_mul(out=ctx_t, in0=ctx_raw, in1=sc_t)
            wf_t = consts.tile([128, NK, D], F32R)
            nc.gpsimd.dma_start(out=wf_t, in_=wf.ap().rearrange("(k p) n -> p k n", p=128))
            res_t = consts.tile([128, NRC, D], F32)
            nc.gpsimd.dma_start(out=res_t, in_=resid.ap().rearrange("(rc p) n -> p rc n", p=128))
            g_bc = consts.tile([128, D], F32)
            nc.gpsimd.dma_start(out=g_bc, in_=ln_g.ap().partition_broadcast(128))
            b_bc = consts.tile([128, D], F32)
            nc.gpsimd.dma_start(out=b_bc, in_=ln_b.ap().partition_broadcast(128))
            eps_t = consts.tile([128, 1], F32)
            nc.vector.memset(eps_t, LN_EPS)

            out_view = normed.ap().rearrange("(rc p) n -> p rc n", p=128)
            for rc in range(NRC):
                x_sb = work.tile([128, D], F32, tag="x", name=f"x{rc}")
                for og in range(NOG):
                    acc = ps.tile([128, 512], F32, tag="ps", name=f"acc{rc}_{og}")
                    for k in range(NK):
                        nc.tensor.matmul(
                            acc,
                            lhsT=ctx_t[:, k, rc * 128:(rc + 1) * 128],
                            rhs=wf_t[:, k, og * 512:(og + 1) * 512],
                            start=(k == 0), stop=(k == NK - 1),
                        )
                    nc.vector.tensor_add(
                        out=x_sb[:, og * 512:(og + 1) * 512],
                        in0=acc, in1=res_t[:, rc, og * 512:(og + 1) * 512],
                    )
                stats = stat.tile([128, 2, 6], F32, tag="st", name=f"st{rc}")
                for sg in range(2):
                    nc.vector.bn_stats(
                        out=stats[:, sg, :], in_=x_sb[:, sg * 512:(sg + 1) * 512]
                    )
                mv = stat.tile([128, 2], F32, tag="mv", name=f"mv{rc}")
                nc.vector.bn_aggr(out=mv, in_=stats)
                sd = stat.tile([128, 1], F32, tag="sd", name=f"sd{rc}")
                nc.scalar.activation(out=sd, in_=mv[:, 1:2], func=SQRT, bias=eps_t)
                rstd = stat.tile([128, 1], F32, tag="rstd", name=f"rst{rc}")
                nc.vector.reciprocal(out=rstd, in_=sd)
                y_sb = work.tile([128, D], F32, tag="y", name=f"y{rc}")
                nc.vector.tensor_scalar(
                    out=y_sb, in0=x_sb, scalar1=mv[:, 0:1], scalar2=rstd,
                    op0=mybir.AluOpType.subtract, op1=mybir.AluOpType.mult,
                )
                nc.vector.tensor_mul(out=y_sb, in0=y_sb, in1=g_bc)
                nc.vector.tensor_add(out=y_sb, in0=y_sb, in1=b_bc)
                nc.gpsimd.dma_start(out=out_view[:, rc, :], in_=y_sb)

    nc.compile()
    return nc


def kernel(key, value, query, Wk, bk, Wv, bv, Wq, bq, Wf, bf, ln_g, ln_b):
    f32 = lambda a: np.ascontiguousarray(np.asarray(a, dtype=np.float32))
    key, value, query = f32(key), f32(value), f32(query)
    Wk, Wv, Wq, Wf = f32(Wk), f32(Wv), f32(Wq), f32(Wf)
    bk, bv, bq, bf, ln_g, ln_b = f32(bk), f32(bv), f32(bq), f32(bf), f32(ln_g), f32(ln_b)

    # transposed activations [D, B*S]; per-core band column slices
    qT = query.reshape(BS, D).T
    kT = key.reshape(BS, D).T
    vT = value.reshape(BS, D).T

    def band_cols(c):
        # group order g: (b=0,m*=2c), (b=0,2c+1), (b=1,2c), (b=1,2c+1)
        cols = []
        for g in range(NG):
            b, mstar = g // BPC, 2 * c + g % BPC
            s0 = b * S + mstar * 128
            cols.append(np.arange(s0, s0 + 128))
        return np.concatenate(cols)

    if "nc1" not in _CACHE:
        _CACHE["nc1"] = _build_kernel1()
    nc1 = _CACHE["nc1"]

    bq2 = np.ascontiguousarray(bq.reshape(H, DH).T)
    bk2 = np.ascontiguousarray(bk.reshape(H, DH).T)
    bv2 = np.ascontiguousarray(bv.reshape(H, DH).T)
    in_maps1 = []
    for c in range(NCORES):
        cols = band_cols(c)
        in_maps1.append({
            "xq": np.ascontiguousarray(qT[:, cols]),
            "xk": np.ascontiguousarray(kT[:, cols]),
            "xv": np.ascontiguousarray(vT[:, cols]),
            "wq": Wq, "wk": Wk, "wv": Wv,
            "bq": bq2, "bk": bk2, "bv": bv2,
        })
    res1 = run_bass_kernel_spmd(nc1, in_maps1, core_ids=list(range(NCORES)))
    _CACHE["res1"] = res1

    attention = np.empty((B * H, S, S), dtype=np.float32)
    ctxT_full = np.empty((D, BS), dtype=np.float32)
    scale_full = np.empty((D, BS), dtype=np.float32)
    for c in range(NCORES):
        r = res1.results[c]
        for g in range(NG):
            b, mstar = g // BPC, 2 * c + g % BPC
            attention[b * H + mstar] = r["attn"][g]
            fr0 = b * S + mstar * 128
            cg = r["ctx_out"][g].reshape(DH, 128, H)         # (i, s', h)
            ctxT_full[:, fr0:fr0 + 128] = (
                cg.transpose(2, 0, 1).reshape(D, 128)        # rows h*64+i
            )
            rsg = r["rs_out"][g].reshape(128, H)             # (s', h)
            scale_full[:, fr0:fr0 + 128] = np.repeat(
                (1.0 / rsg).T, DH, axis=0                    # rows h*64+i
            )

    if "nc2" not in _CACHE:
        _CACHE["nc2"] = _build_kernel2()
    nc2 = _CACHE["nc2"]

    query_flat = query.reshape(BS, D)
    in_maps2 = []
    for r in range(NCORES):
        rows = slice(r * RPC, (r + 1) * RPC)
        in_maps2.append({
            "ctxT_r": np.ascontiguousarray(ctxT_full[:, rows]),
            "scale_r": np.ascontiguousarray(scale_full[:, rows]),
            "wf": Wf,
            "resid": np.ascontiguousarray(query_flat[rows, :] + bf[None, :]),
            "ln_g": ln_g, "ln_b": ln_b,
        })
    res2 = run_bass_kernel_spmd(nc2, in_maps2, core_ids=list(range(NCORES)))
    _CACHE["res2"] = res2

    normed = np.empty((BS, D), dtype=np.float32)
    for r in range(NCORES):
        normed[r * RPC:(r + 1) * RPC, :] = res2.results[r]["normed"]
    normed = normed.reshape(B, S, D)

    return (normed, attention)


# revision 15
# speedup vs baseline: 13.4433x; 1.0030x over previous
"""Trainium2 Bass kernel for nn_MultiHeadAttention_59760174957330.

Shapes: B=2, S=2048, D=1024, H=16, d=64. The reference's
`k.reshape(B*H, S, d)` is a raw memory view (no head transpose), so each
"attention group" m = b*16 + m* is self-attention over a 128-token sequence
band (rows s in [128*m*, 128*(m*+1))) viewed as [2048, 64]: position
t = 16*s' + h carries head h's 64 features at band row s'.

Sharding: band-parallel — core c owns the 4 groups (b in {0,1}) x
(m* in {2c, 2c+1}). Projections are computed per-core for just that core's
512 sequence rows (full D), weights replicated. The output projection +
LayerNorm run as a second data-parallel NEFF over flattened rows after a
host reshard of the small (16 MB) context tensor.

Device pipeline per group (NEFF 1):
  S~^T tiles = k_chunk q^T (PE fp32r, head-major/t~ rows, t-ordered cols via
  strided APs), P^T = exp(0.5 S~^T) (ACT -> bf16),
  ctx = V~^T P^T accumulated in PSUM with a ones-column producing softmax
  row-sums for free; then the output pass A = exp(0.5 S - ln rowsum)
  fuses normalization into the ACT exp and streams straight to DRAM in
  natural (t1, t2) order. Context is normalized producer-side.
NEFF 2: out = context @ Wf + residual -> LayerNorm, 512 rows/core.
"""
import numpy as np

import concourse.bass as bass
import concourse.mybir as mybir
import concourse.tile as tile
from concourse import bacc
from concourse.bass_utils import run_bass_kernel_spmd
from concourse.masks import make_identity

F32 = mybir.dt.float32
F32R = mybir.dt.float32r
BF16 = mybir.dt.bfloat16
FP16 = mybir.dt.float16
EXP = mybir.ActivationFunctionType.Exp
LN_ = mybir.ActivationFunctionType.Ln
SQRT = mybir.ActivationFunctionType.Sqrt

B, S, D, H = 2, 2048, 1024, 16
DH = D // H           # 64
NCORES = 8
BPC = 2               # bands (m*) per core
NG = B * BPC          # 4 groups per core
ROWS = NG * 128       # 512 seq rows per core
BS = B * S
RPC = BS // NCORES    # 512 flat rows per core in NEFF 2
LN_EPS = 1e-5
SCALE = float(DH // H) ** (-0.5)  # 0.5

T = S                 # positions per group (2048 = 128 s' x 16 h)
NKC = T // 128        # 16 t~ chunks (= heads)
NQG = T // 512        # 4
NQC = T // 128        # 16 t1 chunks
NK = D // 128         # 8 contraction chunks

_CACHE = {}


def _build_kernel1():
    nc = bacc.Bacc("TRN2", target_bir_lowering=False, debug=False, num_devices=NCORES)
    xq = nc.dram_tensor("xq", [D, ROWS], F32R, kind="ExternalInput")
    xk = nc.dram_tensor("xk", [D, ROWS], F32R, kind="ExternalInput")
    xv = nc.dram_tensor("xv", [D, ROWS], F32R, kind="ExternalInput")
    wq = nc.dram_tensor("wq", [D, D], F32R, kind="ExternalInput")
    wk = nc.dram_tensor("wk", [D, D], F32R, kind="ExternalInput")
    wv = nc.dram_tensor("wv", [D, D], F32R, kind="ExternalInput")
    bq = nc.dram_tensor("bq", [DH, H], F32, kind="ExternalInput")
    bk = nc.dram_tensor("bk", [DH, H], F32, kind="ExternalInput")
    bv = nc.dram_tensor("bv", [DH, H], F32, kind="ExternalInput")
    attn = nc.dram_tensor("attn", [NG, T, T], F32, kind="ExternalOutput")
    ctx_out = nc.dram_tensor("ctx_out", [NG, DH, T], F32, kind="ExternalOutput")
    rs_out = nc.dram_tensor("rs_out", [NG, T], F32, kind="ExternalOutput")

    with tile.TileContext(nc) as tc:
        with (
            tc.tile_pool(name="consts", bufs=1) as consts,
            tc.tile_pool(name="pt", bufs=3) as ptp,
            tc.tile_pool(name="asb", bufs=3) as asbp,
            tc.tile_pool(name="small", bufs=2) as small,
            tc.tile_pool(name="dram", bufs=4, space="DRAM") as dramp,
            tc.tile_pool(name="ps", bufs=2, space="PSUM") as ps,
            tc.tile_pool(name="ps_acc", bufs=4, space="PSUM") as ps_acc,
        ):
            # t-ordered projected activations: [64, group, t = 16 s' + h]
            q_t = consts.tile([DH, NG, T], BF16)
            k_t = consts.tile([DH, NG, T], BF16)
            # V~ blocks per (group, head): [128 s', 65] with ones at col 64
            v_sb = consts.tile([128, NG, H, DH + 1], BF16)
            nc.vector.memset(v_sb[:, :, :, DH:DH + 1], 1.0)
            ident = consts.tile([DH, DH], F32)
            make_identity(nc, ident)
            ones65 = consts.tile([DH + 1, 1], F32)
            nc.vector.memset(ones65, 1.0)

            # ---------- phase A: projections (M=64 per head) ----------
            with tc.tile_pool(name="pa", bufs=1) as pa:
                for tname, x_in, w_in, b_in in (
                    ("q", xq, wq, bq), ("k", xk, wk, bk), ("v", xv, wv, bv),
                ):
                    x_blk = pa.tile([128, NK, ROWS], F32R, tag="xblk", name=f"xb_{tname}", bufs=2)
                    nc.sync.dma_start(
                        out=x_blk, in_=x_in.ap().rearrange("(kd p) r -> p kd r", p=128)
                    )
                    w_sb = pa.tile([128, NK, D], F32R, tag="wsb", name=f"w_{tname}", bufs=1)
                    nc.sync.dma_start(
                        out=w_sb, in_=w_in.ap().rearrange("(kd p) n -> p kd n", p=128)
                    )
                    b_sb = pa.tile([DH, H], F32, tag="bsb", name=f"b_{tname}", bufs=3)
                    nc.gpsimd.dma_start(out=b_sb, in_=b_in.ap())

                    for h in range(H):
                        acc = ps.tile([DH, ROWS], F32, tag="ps", name=f"pj_{tname}{h}")
                        for kd in range(NK):
                            nc.tensor.matmul(
                                acc,
                                lhsT=w_sb[:, kd, h * DH:(h + 1) * DH],
                                rhs=x_blk[:, kd, :],
                                start=(kd == 0), stop=(kd == NK - 1),
                            )
                        if tname == "v":
                            vT_h = pa.tile([DH, NG, 128], F32, tag="vth", name=f"vth{h}", bufs=2)
                            nc.vector.tensor_scalar_add(
                                out=vT_h,
                                in0=acc.rearrange("p (g s) -> p g s", g=NG),
                                scalar1=b_sb[:, h:h + 1],
                            )
                            for g in range(NG):
                                tr_ps = ps.tile([128, DH], F32, tag="ps", name=f"tr{g}_{h}")
                                nc.tensor.transpose(tr_ps, vT_h[:, g, :], ident)
                                nc.vector.tensor_copy(v_sb[:, g, h, 0:DH], tr_ps)
                        else:
                            dest = {"q": q_t, "k": k_t}[tname]
                            dview = dest.rearrange("p g (s h) -> p g h s", h=H)
                            nc.vector.tensor_scalar_add(
                                out=dview[:, :, h, :],
                                in0=acc.rearrange("p (g s) -> p g s", g=NG),
                                scalar1=b_sb[:, h:h + 1],
                            )

            # head-major view: [p, g, h, s'] with s'-stride 16
            k_hm = k_t.rearrange("p g (s h) -> p g h s", h=H)

            # ---------- phase B: attention per group ----------
            for g in range(NG):
                # B1: S~^T -> exp -> P^T (bf16, rows t~/head-major, cols t1-order)
                ctx_ps = [
                    ps_acc.tile([DH + 1, 512], F32, tag="ctx", name=f"ctx{g}_{i}")
                    for i in range(NQG)
                ]
                for kc in range(NKC):
                    pt_t = ptp.tile([128, T], BF16, tag="pt", name=f"pt{g}_{kc}")
                    for half in range(2):
                        st_ps = ps.tile([128, 1024], F32, tag="ps", name=f"st{g}_{kc}_{half}")
                        for sub in range(2):
                            qg = half * 2 + sub
                            nc.tensor.matmul(
                                st_ps[:, sub * 512:(sub + 1) * 512],
                                lhsT=k_hm[:, g, kc, :],
                                rhs=q_t[:, g, qg * 512:(qg + 1) * 512],
                                start=True, stop=True,
                            )
                        nc.scalar.activation(
                            out=pt_t[:, half * 1024:(half + 1) * 1024], in_=st_ps,
                            func=EXP, scale=SCALE,
                        )
                    for qg in range(NQG):
                        nc.tensor.matmul(
                            ctx_ps[qg],
                            lhsT=v_sb[:, g, kc, :],
                            rhs=pt_t[:, qg * 512:(qg + 1) * 512],
                            start=(kc == 0), stop=(kc == NKC - 1),
                        )

                # evacuate unnormalized context + rowsum row, contiguous outputs
                ctx_sb = small.tile([DH + 1, T], F32, tag="ctxsb", name=f"cs{g}", bufs=2)
                for qg in range(NQG):
                    nc.vector.tensor_copy(
                        ctx_sb[:, qg * 512:(qg + 1) * 512], ctx_ps[qg]
                    )
                nc.sync.dma_start(out=ctx_out.ap()[g, :, :], in_=ctx_sb[0:DH, :])
                nc.sync.dma_start(out=rs_out.ap()[g, :], in_=ctx_sb[DH:DH + 1, :])

                # rowsum -> [128 t1, 16 chunk] via 16 K=1 matmuls (PE transpose)
                rsT_ps = ps.tile([128, NQC], F32, tag="ps", name=f"rsT{g}")
                for c16 in range(NQC):
                    nc.tensor.matmul(
                        rsT_ps[:, c16:c16 + 1],
                        lhsT=ctx_sb[DH:DH + 1, c16 * 128:(c16 + 1) * 128],
                        rhs=ones65[DH:DH + 1, :],
                        start=True, stop=True,
                    )
                recipT = small.tile([128, NQC], F32, tag="recipT", name=f"rcT{g}", bufs=4)
                nc.vector.reciprocal(out=recipT, in_=rsT_ps)
                negln = small.tile([128, NQC], F32, tag="negln", name=f"nl{g}", bufs=4)
                nc.scalar.activation(out=negln, in_=recipT, func=LN_)

                # B2: A = exp(0.5 S - ln rowsum), natural (t1, t2) order
                for qc in range(NQC):
                    a_sb = asbp.tile([128, T], F32, tag="a", name=f"a{g}_{qc}")
                    for half in range(2):
                        s_ps = ps.tile([128, 1024], F32, tag="ps", name=f"s{g}_{qc}_{half}")
                        for sub in range(2):
                            kg = half * 2 + sub
                            nc.tensor.matmul(
                                s_ps[:, sub * 512:(sub + 1) * 512],
                                lhsT=q_t[:, g, qc * 128:(qc + 1) * 128],
                                rhs=k_t[:, g, kg * 512:(kg + 1) * 512],
                                start=True, stop=True,
                            )
                        nc.scalar.activation(
                            out=a_sb[:, half * 1024:(half + 1) * 1024], in_=s_ps,
                            func=EXP, scale=SCALE, bias=negln[:, qc:qc + 1],
                        )
                    nc.sync.dma_start(
                        out=attn.ap()[g, qc * 128:(qc + 1) * 128, :], in_=a_sb
                    )

    nc.compile()
    return nc


def _build_kernel2():
    nc = bacc.Bacc("TRN2", target_bir_lowering=False, debug=False, num_devices=NCORES)
    ctxT_r = nc.dram_tensor("ctxT_r", [D, RPC], F32, kind="ExternalInput")
    scale_r = nc.dram_tensor("scale_r", [D, RPC], F32, kind="ExternalInput")
    wf = nc.dram_tensor("wf", [D, D], F32, kind="ExternalInput")
    resid = nc.dram_tensor("resid", [RPC, D], F32, kind="ExternalInput")
    ln_g = nc.dram_tensor("ln_g", [D], F32, kind="ExternalInput")
    ln_b = nc.dram_tensor("ln_b", [D], F32, kind="ExternalInput")
    normed = nc.dram_tensor("normed", [RPC, D], F32, kind="ExternalOutput")

    NRC = RPC // 128  # 4
    NOG = D // 512    # 2

    with tile.TileContext(nc) as tc:
        with (
            tc.tile_pool(name="consts", bufs=1) as consts,
            tc.tile_pool(name="work", bufs=2) as work,
            tc.tile_pool(name="stat", bufs=4) as stat,
            tc.tile_pool(name="ps", bufs=2, space="PSUM") as ps,
        ):
            ctx_raw = consts.tile([128, NK, RPC], F32)
            nc.sync.dma_start(out=ctx_raw, in_=ctxT_r.ap().rearrange("(k p) r -> p k r", p=128))
            sc_t = consts.tile([128, NK, RPC], F32)
            nc.sync.dma_start(out=sc_t, in_=scale_r.ap().rearrange("(k p) r -> p k r", p=128))
            ctx_t = consts.tile([128, NK, RPC], F32R)
            nc.vector.tensor_mul(out=ctx_t, in0=ctx_raw, in1=sc_t)
            wf_t = consts.tile([128, NK, D], F32R)
            nc.gpsimd.dma_start(out=wf_t, in_=wf.ap().rearrange("(k p) n -> p k n", p=128))
            res_t = consts.tile([128, NRC, D], F32)
            nc.gpsimd.dma_start(out=res_t, in_=resid.ap().rearrange("(rc p) n -> p rc n", p=128))
            g_bc = consts.tile([128, D], F32)
            nc.gpsimd.dma_start(out=g_bc, in_=ln_g.ap().partition_broadcast(128))
            b_bc = consts.tile([128, D], F32)
            nc.gpsimd.dma_start(out=b_bc, in_=ln_b.ap().partition_broadcast(128))
            eps_t = consts.tile([128, 1], F32)
            nc.vector.memset(eps_t, LN_EPS)

            out_view = normed.ap().rearrange("(rc p) n -> p rc n", p=128)
            for rc in range(NRC):
                x_sb = work.tile([128, D], F32, tag="x", name=f"x{rc}")
                for og in range(NOG):
                    acc = ps.tile([128, 512], F32, tag="ps", name=f"acc{rc}_{og}")
                    for k in range(NK):
                        nc.tensor.matmul(
                            acc,
                            lhsT=ctx_t[:, k, rc * 128:(rc + 1) * 128],
                            rhs=wf_t[:, k, og * 512:(og + 1) * 512],
                            start=(k == 0), stop=(k == NK - 1),
                        )
                    nc.vector.tensor_add(
                        out=x_sb[:, og * 512:(og + 1) * 512],
                        in0=acc, in1=res_t[:, rc, og * 512:(og + 1) * 512],
                    )
                stats = stat.tile([128, 2, 6], F32, tag="st", name=f"st{rc}")
                for sg in range(2):
                    nc.vector.bn_stats(
                        out=stats[:, sg, :], in_=x_sb[:, sg * 512:(sg + 1) * 512]
                    )
                mv = stat.tile([128, 2], F32, tag="mv", name=f"mv{rc}")
                nc.vector.bn_aggr(out=mv, in_=stats)
                sd = stat.tile([128, 1], F32, tag="sd", name=f"sd{rc}")
                nc.scalar.activation(out=sd, in_=mv[:, 1:2], func=SQRT, bias=eps_t)
                rstd = stat.tile([128, 1], F32, tag="rstd", name=f"rst{rc}")
                nc.vector.reciprocal(out=rstd, in_=sd)
                y_sb = work.tile([128, D], F32, tag="y", name=f"y{rc}")
                nc.vector.tensor_scalar(
                    out=y_sb, in0=x_sb, scalar1=mv[:, 0:1], scalar2=rstd,
                    op0=mybir.AluOpType.subtract, op1=mybir.AluOpType.mult,
                )
                nc.vector.tensor_mul(out=y_sb, in0=y_sb, in1=g_bc)
                nc.vector.tensor_add(out=y_sb, in0=y_sb, in1=b_bc)
                nc.gpsimd.dma_start(out=out_view[:, rc, :], in_=y_sb)

    nc.compile()
    return nc


def kernel(key, value, query, Wk, bk, Wv, bv, Wq, bq, Wf, bf, ln_g, ln_b):
    f32 = lambda a: np.ascontiguousarray(np.asarray(a, dtype=np.float32))
    key, value, query = f32(key), f32(value), f32(query)
    Wk, Wv, Wq, Wf = f32(Wk), f32(Wv), f32(Wq), f32(Wf)
    bk, bv, bq, bf, ln_g, ln_b = f32(bk), f32(bv), f32(bq), f32(bf), f32(ln_g), f32(ln_b)

    # transposed activations [D, B*S]; per-core band column slices
    qT = query.reshape(BS, D).T
    kT = key.reshape(BS, D).T
    vT = value.reshape(BS, D).T

    def band_cols(c):
        # group order g: (b=0,m*=2c), (b=0,2c+1), (b=1,2c), (b=1,2c+1)
        cols = []
        for g in range(NG):
            b, mstar = g // BPC, 2 * c + g % BPC
            s0 = b * S + mstar * 128
            cols.append(np.arange(s0, s0 + 128))
        return np.concatenate(cols)

    if "nc1" not in _CACHE:
        _CACHE["nc1"] = _build_kernel1()
    nc1 = _CACHE["nc1"]

    bq2 = np.ascontiguousarray(bq.reshape(H, DH).T)
    bk2 = np.ascontiguousarray(bk.reshape(H, DH).T)
    bv2 = np.ascontiguousarray(bv.reshape(H, DH).T)
    in_maps1 = []
    for c in range(NCORES):
        cols = band_cols(c)
        in_maps1.append({
            "xq": np.ascontiguousarray(qT[:, cols]),
            "xk": np.ascontiguousarray(kT[:, cols]),
            "xv": np.ascontiguousarray(vT[:, cols]),
            "wq": Wq, "wk": Wk, "wv": Wv,
            "bq": bq2, "bk": bk2, "bv": bv2,
        })
    res1 = run_bass_kernel_spmd(nc1, in_maps1, core_ids=list(range(NCORES)))
    _CACHE["res1"] = res1

    attention = np.empty((B * H, S, S), dtype=np.float32)
    ctxT_full = np.empty((D, BS), dtype=np.float32)
    scale_full = np.empty((D, BS), dtype=np.float32)
    for c in range(NCORES):
        r = res1.results[c]
        for g in range(NG):
            b, mstar = g // BPC, 2 * c + g % BPC
            attention[b * H + mstar] = r["attn"][g]
            fr0 = b * S + mstar * 128
            cg = r["ctx_out"][g].reshape(DH, 128, H)         # (i, s', h)
            ctxT_full[:, fr0:fr0 + 128] = (
                cg.transpose(2, 0, 1).reshape(D, 128)        # rows h*64+i
            )
            rsg = r["rs_out"][g].reshape(128, H)             # (s', h)
            scale_full[:, fr0:fr0 + 128] = np.repeat(
                (1.0 / rsg).T, DH, axis=0                    # rows h*64+i
            )

    if "nc2" not in _CACHE:
        _CACHE["nc2"] = _build_kernel2()
    nc2 = _CACHE["nc2"]

    query_flat = query.reshape(BS, D)
    in_maps2 = []
    for r in range(NCORES):
        rows = slice(r * RPC, (r + 1) * RPC)
        in_maps2.append({
            "ctxT_r": np.ascontiguousarray(ctxT_full[:, rows]),
            "scale_r": np.ascontiguousarray(scale_full[:, rows]),
            "wf": Wf,
            "resid": np.ascontiguousarray(query_flat[rows, :] + bf[None, :]),
            "ln_g": ln_g, "ln_b": ln_b,
        })
    res2 = run_bass_kernel_spmd(nc2, in_maps2, core_ids=list(range(NCORES)))
    _CACHE["res2"] = res2

    normed = np.empty((BS, D), dtype=np.float32)
    for r in range(NCORES):
        normed[r * RPC:(r + 1) * RPC, :] = res2.results[r]["normed"]
    normed = normed.reshape(B, S, D)

    return (normed, attention)


# revision 19
# speedup vs baseline: 14.6015x; 1.0862x over previous
"""Trainium2 Bass kernel for nn_MultiHeadAttention_59760174957330.

Shapes: B=2, S=2048, D=1024, H=16, d=64. The reference's
`k.reshape(B*H, S, d)` is a raw memory view (no head transpose), so each
"attention group" m = b*16 + m* is self-attention over a 128-token sequence
band (rows s in [128*m*, 128*(m*+1))) viewed as [2048, 64]: position
t = 16*s' + h carries head h's 64 features at band row s'.

Sharding: band-parallel — core c owns the 4 groups (b in {0,1}) x
(m* in {2c, 2c+1}). Projections are computed per-core for just that core's
512 sequence rows (full D), weights replicated. The output projection +
LayerNorm run as a second data-parallel NEFF over flattened rows after a
host reshard of the small (16 MB) context tensor.

Device pipeline per group (NEFF 1):
  S~^T tiles = k_chunk q^T (PE fp32r, head-major/t~ rows, t-ordered cols via
  strided APs), P^T = exp(0.5 S~^T) (ACT -> bf16),
  ctx = V~^T P^T accumulated in PSUM with a ones-column producing softmax
  row-sums for free; then the output pass A = exp(0.5 S - ln rowsum)
  fuses normalization into the ACT exp and streams straight to DRAM in
  natural (t1, t2) order. Context is normalized producer-side.
NEFF 2: out = context @ Wf + residual -> LayerNorm, 512 rows/core.
"""
import numpy as np

import concourse.bass as bass
import concourse.mybir as mybir
import concourse.tile as tile
from concourse import bacc
from concourse.bass_utils import run_bass_kernel_spmd
from concourse.masks import make_identity

F32 = mybir.dt.float32
F32R = mybir.dt.float32r
BF16 = mybir.dt.bfloat16
FP16 = mybir.dt.float16
EXP = mybir.ActivationFunctionType.Exp
LN_ = mybir.ActivationFunctionType.Ln
SQRT = mybir.ActivationFunctionType.Sqrt

B, S, D, H = 2, 2048, 1024, 16
DH = D // H           # 64
NCORES = 8
BPC = 2               # bands (m*) per core
NG = B * BPC          # 4 groups per core
ROWS = NG * 128       # 512 seq rows per core
BS = B * S
RPC = BS // NCORES    # 512 flat rows per core in NEFF 2
LN_EPS = 1e-5
SCALE = float(DH // H) ** (-0.5)  # 0.5

T = S                 # positions per group (2048 = 128 s' x 16 h)
NKC = T // 128        # 16 t~ chunks (= heads)
NQG = T // 512        # 4
NQC = T // 128        # 16 t1 chunks
NK = D // 128         # 8 contraction chunks

_CACHE = {}


def _build_kernel1():
    nc = bacc.Bacc("TRN2", target_bir_lowering=False, debug=False, num_devices=NCORES)
    xq = nc.dram_tensor("xq", [D, ROWS], FP16, kind="ExternalInput")
    xk = nc.dram_tensor("xk", [D, ROWS], FP16, kind="ExternalInput")
    xv = nc.dram_tensor("xv", [D, ROWS], FP16, kind="ExternalInput")
    wq = nc.dram_tensor("wq", [D, D], FP16, kind="ExternalInput")
    wk = nc.dram_tensor("wk", [D, D], FP16, kind="ExternalInput")
    wv = nc.dram_tensor("wv", [D, D], FP16, kind="ExternalInput")
    bq = nc.dram_tensor("bq", [DH, H], F32, kind="ExternalInput")
    bk = nc.dram_tensor("bk", [DH, H], F32, kind="ExternalInput")
    bv = nc.dram_tensor("bv", [DH, H], F32, kind="ExternalInput")
    attn = nc.dram_tensor("attn", [NG, T, T], F32, kind="ExternalOutput")
    ctx_out = nc.dram_tensor("ctx_out", [NG, DH, T], F32, kind="ExternalOutput")
    rs_out = nc.dram_tensor("rs_out", [NG, T], F32, kind="ExternalOutput")

    with tile.TileContext(nc) as tc:
        with (
            tc.tile_pool(name="consts", bufs=1) as consts,
            tc.tile_pool(name="pt", bufs=3) as ptp,
            tc.tile_pool(name="asb", bufs=3) as asbp,
            tc.tile_pool(name="small", bufs=2) as small,
            tc.tile_pool(name="dram", bufs=4, space="DRAM") as dramp,
            tc.tile_pool(name="ps", bufs=2, space="PSUM") as ps,
            tc.tile_pool(name="ps_acc", bufs=4, space="PSUM") as ps_acc,
        ):
            # t-ordered projected activations: [64, group, t = 16 s' + h]
            q_t = consts.tile([DH, NG, T], FP16)
            k_t = consts.tile([DH, NG, T], FP16)
            # V~ blocks per (group, head): [128 s', 65] with ones at col 64
            v_sb = consts.tile([128, NG, H, DH + 1], BF16)
            nc.vector.memset(v_sb[:, :, :, DH:DH + 1], 1.0)
            ident = consts.tile([DH, DH], F32)
            make_identity(nc, ident)
            ones65 = consts.tile([DH + 1, 1], F32)
            nc.vector.memset(ones65, 1.0)

            # ---------- phase A: projections (M=64 per head) ----------
            with tc.tile_pool(name="pa", bufs=1) as pa:
                for tname, x_in, w_in, b_in in (
                    ("q", xq, wq, bq), ("k", xk, wk, bk), ("v", xv, wv, bv),
                ):
                    x_blk = pa.tile([128, NK, ROWS], FP16, tag="xblk", name=f"xb_{tname}", bufs=2)
                    nc.sync.dma_start(
                        out=x_blk, in_=x_in.ap().rearrange("(kd p) r -> p kd r", p=128)
                    )
                    w_sb = pa.tile([128, NK, D], FP16, tag="wsb", name=f"w_{tname}", bufs=2)
                    nc.sync.dma_start(
                        out=w_sb, in_=w_in.ap().rearrange("(kd p) n -> p kd n", p=128)
                    )
                    b_sb = pa.tile([DH, H], F32, tag="bsb", name=f"b_{tname}", bufs=3)
                    nc.gpsimd.dma_start(out=b_sb, in_=b_in.ap())

                    for h in range(H):
                        acc = ps.tile([DH, ROWS], F32, tag="ps", name=f"pj_{tname}{h}")
                        for kd in range(NK):
                            nc.tensor.matmul(
                                acc,
                                lhsT=w_sb[:, kd, h * DH:(h + 1) * DH],
                                rhs=x_blk[:, kd, :],
                                start=(kd == 0), stop=(kd == NK - 1),
                            )
                        if tname == "v":
                            vT_h = pa.tile([DH, NG, 128], F32, tag="vth", name=f"vth{h}", bufs=2)
                            nc.vector.tensor_scalar_add(
                                out=vT_h,
                                in0=acc.rearrange("p (g s) -> p g s", g=NG),
                                scalar1=b_sb[:, h:h + 1],
                            )
                            for g in range(NG):
                                tr_ps = ps.tile([128, DH], F32, tag="ps", name=f"tr{g}_{h}")
                                nc.tensor.transpose(tr_ps, vT_h[:, g, :], ident)
                                nc.vector.tensor_copy(v_sb[:, g, h, 0:DH], tr_ps)
                        else:
                            dest = {"q": q_t, "k": k_t}[tname]
                            dview = dest.rearrange("p g (s h) -> p g h s", h=H)
                            nc.vector.tensor_scalar_add(
                                out=dview[:, :, h, :],
                                in0=acc.rearrange("p (g s) -> p g s", g=NG),
                                scalar1=b_sb[:, h:h + 1],
                            )

            # head-major view: [p, g, h, s'] with s'-stride 16
            k_hm = k_t.rearrange("p g (s h) -> p g h s", h=H)

            # ---------- phase B: attention per group ----------
            for g in range(NG):
                # B1: S~^T -> exp -> P^T (bf16, rows t~/head-major, cols t1-order)
                ctx_ps = [
                    ps_acc.tile([DH + 1, 512], F32, tag="ctx", name=f"ctx{g}_{i}")
                    for i in range(NQG)
                ]
                for kc in range(NKC):
                    pt_t = ptp.tile([128, T], BF16, tag="pt", name=f"pt{g}_{kc}")
                    for half in range(2):
                        st_ps = ps.tile([128, 1024], F32, tag="ps", name=f"st{g}_{kc}_{half}")
                        for sub in range(2):
                            qg = half * 2 + sub
                            nc.tensor.matmul(
                                st_ps[:, sub * 512:(sub + 1) * 512],
                                lhsT=k_hm[:, g, kc, :],
                                rhs=q_t[:, g, qg * 512:(qg + 1) * 512],
                                start=True, stop=True,
                            )
                        nc.scalar.activation(
                            out=pt_t[:, half * 1024:(half + 1) * 1024], in_=st_ps,
                            func=EXP, scale=SCALE,
                        )
                    for qg in range(NQG):
                        nc.tensor.matmul(
                            ctx_ps[qg],
                            lhsT=v_sb[:, g, kc, :],
                            rhs=pt_t[:, qg * 512:(qg + 1) * 512],
                            start=(kc == 0), stop=(kc == NKC - 1),
                        )

                # evacuate unnormalized context + rowsum row, contiguous outputs
                ctx_sb = small.tile([DH + 1, T], F32, tag="ctxsb", name=f"cs{g}", bufs=2)
                for qg in range(NQG):
                    nc.vector.tensor_copy(
                        ctx_sb[:, qg * 512:(qg + 1) * 512], ctx_ps[qg]
                    )
                nc.sync.dma_start(out=ctx_out.ap()[g, :, :], in_=ctx_sb[0:DH, :])
                nc.sync.dma_start(out=rs_out.ap()[g, :], in_=ctx_sb[DH:DH + 1, :])

                # rowsum -> [128 t1, 16 chunk] via 16 K=1 matmuls (PE transpose)
                rsT_ps = ps.tile([128, NQC], F32, tag="ps", name=f"rsT{g}")
                for c16 in range(NQC):
                    nc.tensor.matmul(
                        rsT_ps[:, c16:c16 + 1],
                        lhsT=ctx_sb[DH:DH + 1, c16 * 128:(c16 + 1) * 128],
                        rhs=ones65[DH:DH + 1, :],
                        start=True, stop=True,
                    )
                recipT = small.tile([128, NQC], F32, tag="recipT", name=f"rcT{g}", bufs=4)
                nc.vector.reciprocal(out=recipT, in_=rsT_ps)
                negln = small.tile([128, NQC], F32, tag="negln", name=f"nl{g}", bufs=4)
                nc.scalar.activation(out=negln, in_=recipT, func=LN_)

                # B2: A = exp(0.5 S - ln rowsum), natural (t1, t2) order
                for qc in range(NQC):
                    a_sb = asbp.tile([128, T], F32, tag="a", name=f"a{g}_{qc}")
                    for half in range(2):
                        s_ps = ps.tile([128, 1024], F32, tag="ps", name=f"s{g}_{qc}_{half}")
                        for sub in range(2):
                            kg = half * 2 + sub
                            nc.tensor.matmul(
                                s_ps[:, sub * 512:(sub + 1) * 512],
                                lhsT=q_t[:, g, qc * 128:(qc + 1) * 128],
                                rhs=k_t[:, g, kg * 512:(kg + 1) * 512],
                                start=True, stop=True,
                            )
                        nc.scalar.activation(
                            out=a_sb[:, half * 1024:(half + 1) * 1024], in_=s_ps,
                            func=EXP, scale=SCALE, bias=negln[:, qc:qc + 1],
                        )
                    nc.sync.dma_start(
                        out=attn.ap()[g, qc * 128:(qc + 1) * 128, :], in_=a_sb
                    )

    nc.compile()
    return nc


def _build_kernel2():
    nc = bacc.Bacc("TRN2", target_bir_lowering=False, debug=False, num_devices=NCORES)
    ctxT_r = nc.dram_tensor("ctxT_r", [D, RPC], F32, kind="ExternalInput")
    scale_r = nc.dram_tensor("scale_r", [D, RPC], F32, kind="ExternalInput")
    wf = nc.dram_tensor("wf", [D, D], F32, kind="ExternalInput")
    resid = nc.dram_tensor("resid", [RPC, D], F32, kind="ExternalInput")
    ln_g = nc.dram_tensor("ln_g", [D], F32, kind="ExternalInput")
    ln_b = nc.dram_tensor("ln_b", [D], F32, kind="ExternalInput")
    normed = nc.dram_tensor("normed", [RPC, D], F32, kind="ExternalOutput")

    NRC = RPC // 128  # 4
    NOG = D // 512    # 2

    with tile.TileContext(nc) as tc:
        with (
            tc.tile_pool(name="consts", bufs=1) as consts,
            tc.tile_pool(name="work", bufs=2) as work,
            tc.tile_pool(name="stat", bufs=4) as stat,
            tc.tile_pool(name="ps", bufs=2, space="PSUM") as ps,
        ):
            ctx_raw = consts.tile([128, NK, RPC], F32)
            nc.sync.dma_start(out=ctx_raw, in_=ctxT_r.ap().rearrange("(k p) r -> p k r", p=128))
            sc_t = consts.tile([128, NK, RPC], F32)
            nc.sync.dma_start(out=sc_t, in_=scale_r.ap().rearrange("(k p) r -> p k r", p=128))
            ctx_t = consts.tile([128, NK, RPC], F32R)
            nc.vector.tensor_mul(out=ctx_t, in0=ctx_raw, in1=sc_t)
            wf_t = consts.tile([128, NK, D], F32R)
            nc.gpsimd.dma_start(out=wf_t, in_=wf.ap().rearrange("(k p) n -> p k n", p=128))
            res_t = consts.tile([128, NRC, D], F32)
            nc.gpsimd.dma_start(out=res_t, in_=resid.ap().rearrange("(rc p) n -> p rc n", p=128))
            g_bc = consts.tile([128, D], F32)
            nc.gpsimd.dma_start(out=g_bc, in_=ln_g.ap().partition_broadcast(128))
            b_bc = consts.tile([128, D], F32)
            nc.gpsimd.dma_start(out=b_bc, in_=ln_b.ap().partition_broadcast(128))
            eps_t = consts.tile([128, 1], F32)
            nc.vector.memset(eps_t, LN_EPS)

            out_view = normed.ap().rearrange("(rc p) n -> p rc n", p=128)
            for rc in range(NRC):
                x_sb = work.tile([128, D], F32, tag="x", name=f"x{rc}")
                for og in range(NOG):
                    acc = ps.tile([128, 512], F32, tag="ps", name=f"acc{rc}_{og}")
                    for k in range(NK):
                        nc.tensor.matmul(
                            acc,
                            lhsT=ctx_t[:, k, rc * 128:(rc + 1) * 128],
                            rhs=wf_t[:, k, og * 512:(og + 1) * 512],
                            start=(k == 0), stop=(k == NK - 1),
                        )
                    nc.vector.tensor_add(
                        out=x_sb[:, og * 512:(og + 1) * 512],
                        in0=acc, in1=res_t[:, rc, og * 512:(og + 1) * 512],
                    )
                stats = stat.tile([128, 2, 6], F32, tag="st", name=f"st{rc}")
                for sg in range(2):
                    nc.vector.bn_stats(
                        out=stats[:, sg, :], in_=x_sb[:, sg * 512:(sg + 1) * 512]
                    )
                mv = stat.tile([128, 2], F32, tag="mv", name=f"mv{rc}")
                nc.vector.bn_aggr(out=mv, in_=stats)
                sd = stat.tile([128, 1], F32, tag="sd", name=f"sd{rc}")
                nc.scalar.activation(out=sd, in_=mv[:, 1:2], func=SQRT, bias=eps_t)
                rstd = stat.tile([128, 1], F32, tag="rstd", name=f"rst{rc}")
                nc.vector.reciprocal(out=rstd, in_=sd)
                y_sb = work.tile([128, D], F32, tag="y", name=f"y{rc}")
                nc.vector.tensor_scalar(
                    out=y_sb, in0=x_sb, scalar1=mv[:, 0:1], scalar2=rstd,
                    op0=mybir.AluOpType.subtract, op1=mybir.AluOpType.mult,
                )
                nc.vector.tensor_mul(out=y_sb, in0=y_sb, in1=g_bc)
                nc.vector.tensor_add(out=y_sb, in0=y_sb, in1=b_bc)
                nc.gpsimd.dma_start(out=out_view[:, rc, :], in_=y_sb)

    nc.compile()
    return nc


def kernel(key, value, query, Wk, bk, Wv, bv, Wq, bq, Wf, bf, ln_g, ln_b):
    f32 = lambda a: np.ascontiguousarray(np.asarray(a, dtype=np.float32))
    key, value, query = f32(key), f32(value), f32(query)
    Wk, Wv, Wq, Wf = f32(Wk), f32(Wv), f32(Wq), f32(Wf)
    bk, bv, bq, bf, ln_g, ln_b = f32(bk), f32(bv), f32(bq), f32(bf), f32(ln_g), f32(ln_b)

    # transposed activations [D, B*S] in fp16; per-core band column slices
    qT = query.reshape(BS, D).T.astype(np.float16)
    kT = key.reshape(BS, D).T.astype(np.float16)
    vT = value.reshape(BS, D).T.astype(np.float16)
    Wq16, Wk16, Wv16 = Wq.astype(np.float16), Wk.astype(np.float16), Wv.astype(np.float16)

    def band_cols(c):
        # group order g: (b=0,m*=2c), (b=0,2c+1), (b=1,2c), (b=1,2c+1)
        cols = []
        for g in range(NG):
            b, mstar = g // BPC, 2 * c + g % BPC
            s0 = b * S + mstar * 128
            cols.append(np.arange(s0, s0 + 128))
        return np.concatenate(cols)

    if "nc1" not in _CACHE:
        _CACHE["nc1"] = _build_kernel1()
    nc1 = _CACHE["nc1"]

    bq2 = np.ascontiguousarray(bq.reshape(H, DH).T)
    bk2 = np.ascontiguousarray(bk.reshape(H, DH).T)
    bv2 = np.ascontiguousarray(bv.reshape(H, DH).T)
    in_maps1 = []
    for c in range(NCORES):
        cols = band_cols(c)
        in_maps1.append({
            "xq": np.ascontiguousarray(qT[:, cols]),
            "xk": np.ascontiguousarray(kT[:, cols]),
            "xv": np.ascontiguousarray(vT[:, cols]),
            "wq": Wq16, "wk": Wk16, "wv": Wv16,
            "bq": bq2, "bk": bk2, "bv": bv2,
        })
    res1 = run_bass_kernel_spmd(nc1, in_maps1, core_ids=list(range(NCORES)))
    _CACHE["res1"] = res1

    attention = np.empty((B * H, S, S), dtype=np.float32)
    ctxT_full = np.empty((D, BS), dtype=np.float32)
    scale_full = np.empty((D, BS), dtype=np.float32)
    for c in range(NCORES):
        r = res1.results[c]
        for g in range(NG):
            b, mstar = g // BPC, 2 * c + g % BPC
            attention[b * H + mstar] = r["attn"][g]
            fr0 = b * S + mstar * 128
            cg = r["ctx_out"][g].reshape(DH, 128, H)         # (i, s', h)
            ctxT_full[:, fr0:fr0 + 128] = (
                cg.transpose(2, 0, 1).reshape(D, 128)        # rows h*64+i
            )
            rsg = r["rs_out"][g].reshape(128, H)             # (s', h)
            scale_full[:, fr0:fr0 + 128] = np.repeat(
                (1.0 / rsg).T, DH, axis=0                    # rows h*64+i
            )

    if "nc2" not in _CACHE:
        _CACHE["nc2"] = _build_kernel2()
    nc2 = _CACHE["nc2"]

    query_flat = query.reshape(BS, D)
    in_maps2 = []
    for r in range(NCORES):
        rows = slice(r * RPC, (r + 1) * RPC)
        in_maps2.append({
            "ctxT_r": np.ascontiguousarray(ctxT_full[:, rows]),
            "scale_r": np.ascontiguousarray(scale_full[:, rows]),
            "wf": Wf,
            "resid": np.ascontiguousarray(query_flat[rows, :] + bf[None, :]),
            "ln_g": ln_g, "ln_b": ln_b,
        })
    res2 = run_bass_kernel_spmd(nc2, in_maps2, core_ids=list(range(NCORES)))
    _CACHE["res2"] = res2

    normed = np.empty((BS, D), dtype=np.float32)
    for r in range(NCORES):
        normed[r * RPC:(r + 1) * RPC, :] = res2.results[r]["normed"]
    normed = normed.reshape(B, S, D)

    return (normed, attention)


# revision 20
# speedup vs baseline: 15.3818x; 1.0534x over previous
"""Trainium2 Bass kernel for nn_MultiHeadAttention_59760174957330.

Shapes: B=2, S=2048, D=1024, H=16, d=64. The reference's
`k.reshape(B*H, S, d)` is a raw memory view (no head transpose), so each
"attention group" m = b*16 + m* is self-attention over a 128-token sequence
band (rows s in [128*m*, 128*(m*+1))) viewed as [2048, 64]: position
t = 16*s' + h carries head h's 64 features at band row s'.

Sharding: band-parallel — core c owns the 4 groups (b in {0,1}) x
(m* in {2c, 2c+1}). Projections are computed per-core for just that core's
512 sequence rows (full D), weights replicated. The output projection +
LayerNorm run as a second data-parallel NEFF over flattened rows after a
host reshard of the small (16 MB) context tensor.

Device pipeline per group (NEFF 1):
  S~^T tiles = k_chunk q^T (PE fp32r, head-major/t~ rows, t-ordered cols via
  strided APs), P^T = exp(0.5 S~^T) (ACT -> bf16),
  ctx = V~^T P^T accumulated in PSUM with a ones-column producing softmax
  row-sums for free; then the output pass A = exp(0.5 S - ln rowsum)
  fuses normalization into the ACT exp and streams straight to DRAM in
  natural (t1, t2) order. Context is normalized producer-side.
NEFF 2: out = context @ Wf + residual -> LayerNorm, 512 rows/core.
"""
import numpy as np

import concourse.bass as bass
import concourse.mybir as mybir
import concourse.tile as tile
from concourse import bacc
from concourse.bass_utils import run_bass_kernel_spmd
from concourse.masks import make_identity

F32 = mybir.dt.float32
F32R = mybir.dt.float32r
BF16 = mybir.dt.bfloat16
FP16 = mybir.dt.float16
EXP = mybir.ActivationFunctionType.Exp
LN_ = mybir.ActivationFunctionType.Ln
SQRT = mybir.ActivationFunctionType.Sqrt

B, S, D, H = 2, 2048, 1024, 16
DH = D // H           # 64
NCORES = 8
BPC = 2               # bands (m*) per core
NG = B * BPC          # 4 groups per core
ROWS = NG * 128       # 512 seq rows per core
BS = B * S
RPC = BS // NCORES    # 512 flat rows per core in NEFF 2
LN_EPS = 1e-5
SCALE = float(DH // H) ** (-0.5)  # 0.5

T = S                 # positions per group (2048 = 128 s' x 16 h)
NKC = T // 128        # 16 t~ chunks (= heads)
NQG = T // 512        # 4
NQC = T // 128        # 16 t1 chunks
NK = D // 128         # 8 contraction chunks

_CACHE = {}


def _build_kernel1():
    nc = bacc.Bacc("TRN2", target_bir_lowering=False, debug=False, num_devices=NCORES)
    xq = nc.dram_tensor("xq", [D, ROWS], FP16, kind="ExternalInput")
    xk = nc.dram_tensor("xk", [D, ROWS], FP16, kind="ExternalInput")
    xv = nc.dram_tensor("xv", [D, ROWS], FP16, kind="ExternalInput")
    wq = nc.dram_tensor("wq", [D, D], FP16, kind="ExternalInput")
    wk = nc.dram_tensor("wk", [D, D], FP16, kind="ExternalInput")
    wv = nc.dram_tensor("wv", [D, D], FP16, kind="ExternalInput")
    bq = nc.dram_tensor("bq", [DH, H], F32, kind="ExternalInput")
    bk = nc.dram_tensor("bk", [DH, H], F32, kind="ExternalInput")
    bv = nc.dram_tensor("bv", [DH, H], F32, kind="ExternalInput")
    attn = nc.dram_tensor("attn", [NG, T, T], F32, kind="ExternalOutput")
    ctx_out = nc.dram_tensor("ctx_out", [NG, DH, T], F32, kind="ExternalOutput")
    rs_out = nc.dram_tensor("rs_out", [NG, T], F32, kind="ExternalOutput")

    with tile.TileContext(nc) as tc:
        with (
            tc.tile_pool(name="consts", bufs=1) as consts,
            tc.tile_pool(name="pt", bufs=3) as ptp,
            tc.tile_pool(name="asb", bufs=3) as asbp,
            tc.tile_pool(name="small", bufs=2) as small,
            tc.tile_pool(name="dram", bufs=4, space="DRAM") as dramp,
            tc.tile_pool(name="ps", bufs=2, space="PSUM") as ps,
            tc.tile_pool(name="ps_acc", bufs=4, space="PSUM") as ps_acc,
        ):
            # t-ordered projected activations: [64, group, t = 16 s' + h]
            q2 = consts.tile([128, NG, T], FP16)
            k2 = consts.tile([128, NG, T], FP16)
            q_t = q2[0:DH, :, :]
            k_t = k2[0:DH, :, :]
            # V~ blocks per (group, head): [128 s', 65] with ones at col 64
            v_sb = consts.tile([128, NG, H, DH + 1], BF16)
            nc.vector.memset(v_sb[:, :, :, DH:DH + 1], 1.0)
            ident = consts.tile([DH, DH], F32)
            make_identity(nc, ident)
            ones65 = consts.tile([DH + 1, 1], F32)
            nc.vector.memset(ones65, 1.0)

            # ---------- phase A: projections (M=64 per head) ----------
            with tc.tile_pool(name="pa", bufs=1) as pa:
                for tname, x_in, w_in, b_in in (
                    ("q", xq, wq, bq), ("k", xk, wk, bk), ("v", xv, wv, bv),
                ):
                    x_blk = pa.tile([128, NK, ROWS], FP16, tag="xblk", name=f"xb_{tname}", bufs=2)
                    nc.sync.dma_start(
                        out=x_blk, in_=x_in.ap().rearrange("(kd p) r -> p kd r", p=128)
                    )
                    w_sb = pa.tile([128, NK, D], FP16, tag="wsb", name=f"w_{tname}", bufs=2)
                    nc.sync.dma_start(
                        out=w_sb, in_=w_in.ap().rearrange("(kd p) n -> p kd n", p=128)
                    )
                    b_sb = pa.tile([DH, H], F32, tag="bsb", name=f"b_{tname}", bufs=3)
                    nc.gpsimd.dma_start(out=b_sb, in_=b_in.ap())

                    for h in range(H):
                        acc = ps.tile([DH, ROWS], F32, tag="ps", name=f"pj_{tname}{h}")
                        for kd in range(NK):
                            nc.tensor.matmul(
                                acc,
                                lhsT=w_sb[:, kd, h * DH:(h + 1) * DH],
                                rhs=x_blk[:, kd, :],
                                start=(kd == 0), stop=(kd == NK - 1),
                            )
                        if tname == "v":
                            vT_h = pa.tile([DH, NG, 128], F32, tag="vth", name=f"vth{h}", bufs=2)
                            nc.vector.tensor_scalar_add(
                                out=vT_h,
                                in0=acc.rearrange("p (g s) -> p g s", g=NG),
                                scalar1=b_sb[:, h:h + 1],
                            )
                            for g in range(NG):
                                tr_ps = ps.tile([128, DH], F32, tag="ps", name=f"tr{g}_{h}")
                                nc.tensor.transpose(tr_ps, vT_h[:, g, :], ident)
                                nc.vector.tensor_copy(v_sb[:, g, h, 0:DH], tr_ps)
                        else:
                            dest = {"q": q_t, "k": k_t}[tname]
                            dview = dest.rearrange("p g (s h) -> p g h s", h=H)
                            nc.vector.tensor_scalar_add(
                                out=dview[:, :, h, :],
                                in0=acc.rearrange("p (g s) -> p g s", g=NG),
                                scalar1=b_sb[:, h:h + 1],
                            )

            # duplicate q/k into partitions 64-127 for paired quadrant matmuls
            nc.gpsimd.dma_start(out=q2[DH:128, :, :], in_=q2[0:DH, :, :])
            nc.gpsimd.dma_start(out=k2[DH:128, :, :], in_=k2[0:DH, :, :])
            # head-major views: [p, g, h, s'] with s'-stride 16
            k_hm = k_t.rearrange("p g (s h) -> p g h s", h=H)
            k_hm2 = k2.rearrange("p g (s h) -> p g h s", h=H)

            # ---------- phase B: attention per group ----------
            for g in range(NG):
                # B1: S~^T -> exp -> P^T (bf16, rows t~/head-major, cols t1-order)
                ctx_ps = [
                    ps_acc.tile([DH + 1, 512], F32, tag="ctx", name=f"ctx{g}_{i}")
                    for i in range(NQG)
                ]
                for kc in range(NKC):
                    pt_t = ptp.tile([128, T], BF16, tag="pt", name=f"pt{g}_{kc}")
                    for half in range(2):
                        st_ps = ps.tile([128, 1024], F32, tag="ps", name=f"st{g}_{kc}_{half}")
                        for sub in range(2):
                            qg = half * 2 + sub
                            nc.tensor.matmul(
                                st_ps[0:DH, sub * 512:(sub + 1) * 512],
                                lhsT=k_hm[:, g, kc, 0:DH],
                                rhs=q_t[:, g, qg * 512:(qg + 1) * 512],
                                start=True, stop=True,
                                tile_position=(0, 0),
                            )
                            nc.tensor.matmul(
                                st_ps[DH:128, sub * 512:(sub + 1) * 512],
                                lhsT=k_hm2[DH:128, g, kc, DH:128],
                                rhs=q2[DH:128, g, qg * 512:(qg + 1) * 512],
                                start=True, stop=True,
                                tile_position=(64, 64),
                            )
                        nc.scalar.activation(
                            out=pt_t[:, half * 1024:(half + 1) * 1024], in_=st_ps,
                            func=EXP, scale=SCALE,
                        )
                    for qg in range(NQG):
                        nc.tensor.matmul(
                            ctx_ps[qg],
                            lhsT=v_sb[:, g, kc, :],
                            rhs=pt_t[:, qg * 512:(qg + 1) * 512],
                            start=(kc == 0), stop=(kc == NKC - 1),
                        )

                # evacuate unnormalized context + rowsum row, contiguous outputs
                ctx_sb = small.tile([DH + 1, T], F32, tag="ctxsb", name=f"cs{g}", bufs=2)
                for qg in range(NQG):
                    nc.vector.tensor_copy(
                        ctx_sb[:, qg * 512:(qg + 1) * 512], ctx_ps[qg]
                    )
                nc.sync.dma_start(out=ctx_out.ap()[g, :, :], in_=ctx_sb[0:DH, :])
                nc.sync.dma_start(out=rs_out.ap()[g, :], in_=ctx_sb[DH:DH + 1, :])

                # rowsum -> [128 t1, 16 chunk] via 16 K=1 matmuls (PE transpose)
                rsT_ps = ps.tile([128, NQC], F32, tag="ps", name=f"rsT{g}")
                for c16 in range(NQC):
                    nc.tensor.matmul(
                        rsT_ps[:, c16:c16 + 1],
                        lhsT=ctx_sb[DH:DH + 1, c16 * 128:(c16 + 1) * 128],
                        rhs=ones65[DH:DH + 1, :],
                        start=True, stop=True,
                    )
                recipT = small.tile([128, NQC], F32, tag="recipT", name=f"rcT{g}", bufs=4)
                nc.vector.reciprocal(out=recipT, in_=rsT_ps)
                negln = small.tile([128, NQC], F32, tag="negln", name=f"nl{g}", bufs=4)
                nc.scalar.activation(out=negln, in_=recipT, func=LN_)

                # B2: A = exp(0.5 S - ln rowsum), natural (t1, t2) order
                for qc in range(NQC):
                    a_sb = asbp.tile([128, T], F32, tag="a", name=f"a{g}_{qc}")
                    for half in range(2):
                        s_ps = ps.tile([128, 1024], F32, tag="ps", name=f"s{g}_{qc}_{half}")
                        for sub in range(2):
                            kg = half * 2 + sub
                            nc.tensor.matmul(
                                s_ps[0:DH, sub * 512:(sub + 1) * 512],
                                lhsT=q_t[:, g, qc * 128:qc * 128 + DH],
                                rhs=k_t[:, g, kg * 512:(kg + 1) * 512],
                                start=True, stop=True,
                                tile_position=(0, 0),
                            )
                            nc.tensor.matmul(
                                s_ps[DH:128, sub * 512:(sub + 1) * 512],
                                lhsT=q2[DH:128, g, qc * 128 + DH:(qc + 1) * 128],
                                rhs=k2[DH:128, g, kg * 512:(kg + 1) * 512],
                                start=True, stop=True,
                                tile_position=(64, 64),
                            )
                        nc.scalar.activation(
                            out=a_sb[:, half * 1024:(half + 1) * 1024], in_=s_ps,
                            func=EXP, scale=SCALE, bias=negln[:, qc:qc + 1],
                        )
                    nc.sync.dma_start(
                        out=attn.ap()[g, qc * 128:(qc + 1) * 128, :], in_=a_sb
                    )

    nc.compile()
    return nc


def _build_kernel2():
    nc = bacc.Bacc("TRN2", target_bir_lowering=False, debug=False, num_devices=NCORES)
    ctxT_r = nc.dram_tensor("ctxT_r", [D, RPC], F32, kind="ExternalInput")
    scale_r = nc.dram_tensor("scale_r", [D, RPC], F32, kind="ExternalInput")
    wf = nc.dram_tensor("wf", [D, D], F32, kind="ExternalInput")
    resid = nc.dram_tensor("resid", [RPC, D], F32, kind="ExternalInput")
    ln_g = nc.dram_tensor("ln_g", [D], F32, kind="ExternalInput")
    ln_b = nc.dram_tensor("ln_b", [D], F32, kind="ExternalInput")
    normed = nc.dram_tensor("normed", [RPC, D], F32, kind="ExternalOutput")

    NRC = RPC // 128  # 4
    NOG = D // 512    # 2

    with tile.TileContext(nc) as tc:
        with (
            tc.tile_pool(name="consts", bufs=1) as consts,
            tc.tile_pool(name="work", bufs=2) as work,
            tc.tile_pool(name="stat", bufs=4) as stat,
            tc.tile_pool(name="ps", bufs=2, space="PSUM") as ps,
        ):
            ctx_raw = consts.tile([128, NK, RPC], F32)
            nc.sync.dma_start(out=ctx_raw, in_=ctxT_r.ap().rearrange("(k p) r -> p k r", p=128))
            sc_t = consts.tile([128, NK, RPC], F32)
            nc.sync.dma_start(out=sc_t, in_=scale_r.ap().rearrange("(k p) r -> p k r", p=128))
            ctx_t = consts.tile([128, NK, RPC], F32R)
            nc.vector.tensor_mul(out=ctx_t, in0=ctx_raw, in1=sc_t)
            wf_t = consts.tile([128, NK, D], F32R)
            nc.gpsimd.dma_start(out=wf_t, in_=wf.ap().rearrange("(k p) n -> p k n", p=128))
            res_t = consts.tile([128, NRC, D], F32)
            nc.gpsimd.dma_start(out=res_t, in_=resid.ap().rearrange("(rc p) n -> p rc n", p=128))
            g_bc = consts.tile([128, D], F32)
            nc.gpsimd.dma_start(out=g_bc, in_=ln_g.ap().partition_broadcast(128))
            b_bc = consts.tile([128, D], F32)
            nc.gpsimd.dma_start(out=b_bc, in_=ln_b.ap().partition_broadcast(128))
            eps_t = consts.tile([128, 1], F32)
            nc.vector.memset(eps_t, LN_EPS)

            out_view = normed.ap().rearrange("(rc p) n -> p rc n", p=128)
            for rc in range(NRC):
                x_sb = work.tile([128, D], F32, tag="x", name=f"x{rc}")
                for og in range(NOG):
                    acc = ps.tile([128, 512], F32, tag="ps", name=f"acc{rc}_{og}")
                    for k in range(NK):
                        nc.tensor.matmul(
                            acc,
                            lhsT=ctx_t[:, k, rc * 128:(rc + 1) * 128],
                            rhs=wf_t[:, k, og * 512:(og + 1) * 512],
                            start=(k == 0), stop=(k == NK - 1),
                        )
                    nc.vector.tensor_add(
                        out=x_sb[:, og * 512:(og + 1) * 512],
                        in0=acc, in1=res_t[:, rc, og * 512:(og + 1) * 512],
                    )
                stats = stat.tile([128, 2, 6], F32, tag="st", name=f"st{rc}")
                for sg in range(2):
                    nc.vector.bn_stats(
                        out=stats[:, sg, :], in_=x_sb[:, sg * 512:(sg + 1) * 512]
                    )
                mv = stat.tile([128, 2], F32, tag="mv", name=f"mv{rc}")
                nc.vector.bn_aggr(out=mv, in_=stats)
                sd = stat.tile([128, 1], F32, tag="sd", name=f"sd{rc}")
                nc.scalar.activation(out=sd, in_=mv[:, 1:2], func=SQRT, bias=eps_t)
                rstd = stat.tile([128, 1], F32, tag="rstd", name=f"rst{rc}")
                nc.vector.reciprocal(out=rstd, in_=sd)
                y_sb = work.tile([128, D], F32, tag="y", name=f"y{rc}")
                nc.vector.tensor_scalar(
                    out=y_sb, in0=x_sb, scalar1=mv[:, 0:1], scalar2=rstd,
                    op0=mybir.AluOpType.subtract, op1=mybir.AluOpType.mult,
                )
                nc.vector.tensor_mul(out=y_sb, in0=y_sb, in1=g_bc)
                nc.vector.tensor_add(out=y_sb, in0=y_sb, in1=b_bc)
                nc.gpsimd.dma_start(out=out_view[:, rc, :], in_=y_sb)

    nc.compile()
    return nc


def kernel(key, value, query, Wk, bk, Wv, bv, Wq, bq, Wf, bf, ln_g, ln_b):
    f32 = lambda a: np.ascontiguousarray(np.asarray(a, dtype=np.float32))
    key, value, query = f32(key), f32(value), f32(query)
    Wk, Wv, Wq, Wf = f32(Wk), f32(Wv), f32(Wq), f32(Wf)
    bk, bv, bq, bf, ln_g, ln_b = f32(bk), f32(bv), f32(bq), f32(bf), f32(ln_g), f32(ln_b)

    # transposed activations [D, B*S] in fp16; per-core band column slices
    qT = query.reshape(BS, D).T.astype(np.float16)
    kT = key.reshape(BS, D).T.astype(np.float16)
    vT = value.reshape(BS, D).T.astype(np.float16)
    Wq16, Wk16, Wv16 = Wq.astype(np.float16), Wk.astype(np.float16), Wv.astype(np.float16)

    def band_cols(c):
        # group order g: (b=0,m*=2c), (b=0,2c+1), (b=1,2c), (b=1,2c+1)
        cols = []
        for g in range(NG):
            b, mstar = g // BPC, 2 * c + g % BPC
            s0 = b * S + mstar * 128
            cols.append(np.arange(s0, s0 + 128))
        return np.concatenate(cols)

    if "nc1" not in _CACHE:
        _CACHE["nc1"] = _build_kernel1()
    nc1 = _CACHE["nc1"]

    bq2 = np.ascontiguousarray(bq.reshape(H, DH).T)
    bk2 = np.ascontiguousarray(bk.reshape(H, DH).T)
    bv2 = np.ascontiguousarray(bv.reshape(H, DH).T)
    in_maps1 = []
    for c in range(NCORES):
        cols = band_cols(c)
        in_maps1.append({
            "xq": np.ascontiguousarray(qT[:, cols]),
            "xk": np.ascontiguousarray(kT[:, cols]),
            "xv": np.ascontiguousarray(vT[:, cols]),
            "wq": Wq16, "wk": Wk16, "wv": Wv16,
            "bq": bq2, "bk": bk2, "bv": bv2,
        })
    res1 = run_bass_kernel_spmd(nc1, in_maps1, core_ids=list(range(NCORES)))
    _CACHE["res1"] = res1

    attention = np.empty((B * H, S, S), dtype=np.float32)
    ctxT_full = np.empty((D, BS), dtype=np.float32)
    scale_full = np.empty((D, BS), dtype=np.float32)
    for c in range(NCORES):
        r = res1.results[c]
        for g in range(NG):
            b, mstar = g // BPC, 2 * c + g % BPC
            attention[b * H + mstar] = r["attn"][g]
            fr0 = b * S + mstar * 128
            cg = r["ctx_out"][g].reshape(DH, 128, H)         # (i, s', h)
            ctxT_full[:, fr0:fr0 + 128] = (
                cg.transpose(2, 0, 1).reshape(D, 128)        # rows h*64+i
            )
            rsg = r["rs_out"][g].reshape(128, H)             # (s', h)
            scale_full[:, fr0:fr0 + 128] = np.repeat(
                (1.0 / rsg).T, DH, axis=0                    # rows h*64+i
            )

    if "nc2" not in _CACHE:
        _CACHE["nc2"] = _build_kernel2()
    nc2 = _CACHE["nc2"]

    query_flat = query.reshape(BS, D)
    in_maps2 = []
    for r in range(NCORES):
        rows = slice(r * RPC, (r + 1) * RPC)
        in_maps2.append({
            "ctxT_r": np.ascontiguousarray(ctxT_full[:, rows]),
            "scale_r": np.ascontiguousarray(scale_full[:, rows]),
            "wf": Wf,
            "resid": np.ascontiguousarray(query_flat[rows, :] + bf[None, :]),
            "ln_g": ln_g, "ln_b": ln_b,
        })
    res2 = run_bass_kernel_spmd(nc2, in_maps2, core_ids=list(range(NCORES)))
    _CACHE["res2"] = res2

    normed = np.empty((BS, D), dtype=np.float32)
    for r in range(NCORES):
        normed[r * RPC:(r + 1) * RPC, :] = res2.results[r]["normed"]
    normed = normed.reshape(B, S, D)

    return (normed, attention)


# revision 21
# speedup vs baseline: 16.4013x; 1.0663x over previous
"""Trainium2 Bass kernel for nn_MultiHeadAttention_59760174957330.

Shapes: B=2, S=2048, D=1024, H=16, d=64. The reference's
`k.reshape(B*H, S, d)` is a raw memory view (no head transpose), so each
"attention group" m = b*16 + m* is self-attention over a 128-token sequence
band (rows s in [128*m*, 128*(m*+1))) viewed as [2048, 64]: position
t = 16*s' + h carries head h's 64 features at band row s'.

Sharding: band-parallel — core c owns the 4 groups (b in {0,1}) x
(m* in {2c, 2c+1}). Projections are computed per-core for just that core's
512 sequence rows (full D), weights replicated. The output projection +
LayerNorm run as a second data-parallel NEFF over flattened rows after a
host reshard of the small (16 MB) context tensor.

Device pipeline per group (NEFF 1):
  S~^T tiles = k_chunk q^T (PE fp32r, head-major/t~ rows, t-ordered cols via
  strided APs), P^T = exp(0.5 S~^T) (ACT -> bf16),
  ctx = V~^T P^T accumulated in PSUM with a ones-column producing softmax
  row-sums for free; then the output pass A = exp(0.5 S - ln rowsum)
  fuses normalization into the ACT exp and streams straight to DRAM in
  natural (t1, t2) order. Context is normalized producer-side.
NEFF 2: out = context @ Wf + residual -> LayerNorm, 512 rows/core.
"""
import numpy as np

import concourse.bass as bass
import concourse.mybir as mybir
import concourse.tile as tile
from concourse import bacc
from concourse.bass_utils import run_bass_kernel_spmd
from concourse.masks import make_identity

F32 = mybir.dt.float32
F32R = mybir.dt.float32r
BF16 = mybir.dt.bfloat16
FP16 = mybir.dt.float16
EXP = mybir.ActivationFunctionType.Exp
LN_ = mybir.ActivationFunctionType.Ln
SQRT = mybir.ActivationFunctionType.Sqrt

B, S, D, H = 2, 2048, 1024, 16
DH = D // H           # 64
NCORES = 8
BPC = 2               # bands (m*) per core
NG = B * BPC          # 4 groups per core
ROWS = NG * 128       # 512 seq rows per core
BS = B * S
RPC = BS // NCORES    # 512 flat rows per core in NEFF 2
LN_EPS = 1e-5
SCALE = float(DH // H) ** (-0.5)  # 0.5

T = S                 # positions per group (2048 = 128 s' x 16 h)
NKC = T // 128        # 16 t~ chunks (= heads)
NQG = T // 512        # 4
NQC = T // 128        # 16 t1 chunks
NK = D // 128         # 8 contraction chunks

_CACHE = {}


def _build_kernel1():
    nc = bacc.Bacc("TRN2", target_bir_lowering=False, debug=False, num_devices=NCORES)
    xq = nc.dram_tensor("xq", [D, ROWS], FP16, kind="ExternalInput")
    xk = nc.dram_tensor("xk", [D, ROWS], FP16, kind="ExternalInput")
    xv = nc.dram_tensor("xv", [D, ROWS], FP16, kind="ExternalInput")
    wq = nc.dram_tensor("wq", [D, D], FP16, kind="ExternalInput")
    wk = nc.dram_tensor("wk", [D, D], FP16, kind="ExternalInput")
    wv = nc.dram_tensor("wv", [D, D], FP16, kind="ExternalInput")
    bq = nc.dram_tensor("bq", [DH, H], F32, kind="ExternalInput")
    bk = nc.dram_tensor("bk", [DH, H], F32, kind="ExternalInput")
    bv = nc.dram_tensor("bv", [DH, H], F32, kind="ExternalInput")
    attn = nc.dram_tensor("attn", [NG, T, T], F32, kind="ExternalOutput")
    ctx_out = nc.dram_tensor("ctx_out", [NG, DH, T], F32, kind="ExternalOutput")
    rs_out = nc.dram_tensor("rs_out", [NG, T], F32, kind="ExternalOutput")

    with tile.TileContext(nc) as tc:
        with (
            tc.tile_pool(name="consts", bufs=1) as consts,
            tc.tile_pool(name="pt", bufs=3) as ptp,
            tc.tile_pool(name="asb", bufs=3) as asbp,
            tc.tile_pool(name="small", bufs=2) as small,
            tc.tile_pool(name="dram", bufs=4, space="DRAM") as dramp,
            tc.tile_pool(name="ps", bufs=2, space="PSUM") as ps,
            tc.tile_pool(name="ps_acc", bufs=4, space="PSUM") as ps_acc,
        ):
            # t-ordered projected activations: [64, group, t = 16 s' + h]
            q2 = consts.tile([128, NG, T], FP16)
            k2 = consts.tile([128, NG, T], FP16)
            q_t = q2[0:DH, :, :]
            k_t = k2[0:DH, :, :]
            # V~ blocks per (group, head): [128 s', 65] with ones at col 64
            v_sb = consts.tile([128, NG, H, DH + 1], BF16)
            nc.vector.memset(v_sb[:, :, :, DH:DH + 1], 1.0)
            ident = consts.tile([DH, DH], F32)
            make_identity(nc, ident)
            ones65 = consts.tile([DH + 1, 1], F32)
            nc.vector.memset(ones65, 1.0)

            # ---------- phase A: projections (M=64 per head) ----------
            with tc.tile_pool(name="pa", bufs=1) as pa:
                for tname, x_in, w_in, b_in in (
                    ("q", xq, wq, bq), ("k", xk, wk, bk), ("v", xv, wv, bv),
                ):
                    x_blk = pa.tile([128, NK, ROWS], FP16, tag="xblk", name=f"xb_{tname}", bufs=2)
                    nc.sync.dma_start(
                        out=x_blk, in_=x_in.ap().rearrange("(kd p) r -> p kd r", p=128)
                    )
                    w_sb = pa.tile([128, NK, D], FP16, tag="wsb", name=f"w_{tname}", bufs=2)
                    nc.sync.dma_start(
                        out=w_sb, in_=w_in.ap().rearrange("(kd p) n -> p kd n", p=128)
                    )
                    b_sb = pa.tile([DH, H], F32, tag="bsb", name=f"b_{tname}", bufs=3)
                    nc.gpsimd.dma_start(out=b_sb, in_=b_in.ap())

                    for h in range(H):
                        acc = ps.tile([DH, ROWS], F32, tag="ps", name=f"pj_{tname}{h}")
                        for kd in range(NK):
                            nc.tensor.matmul(
                                acc,
                                lhsT=w_sb[:, kd, h * DH:(h + 1) * DH],
                                rhs=x_blk[:, kd, :],
                                start=(kd == 0), stop=(kd == NK - 1),
                            )
                        if tname == "v":
                            vT_h = pa.tile([DH, NG, 128], F32, tag="vth", name=f"vth{h}", bufs=2)
                            nc.vector.tensor_scalar_add(
                                out=vT_h,
                                in0=acc.rearrange("p (g s) -> p g s", g=NG),
                                scalar1=b_sb[:, h:h + 1],
                            )
                            for g in range(NG):
                                tr_ps = ps.tile([128, DH], F32, tag="ps", name=f"tr{g}_{h}")
                                nc.tensor.transpose(tr_ps, vT_h[:, g, :], ident)
                                nc.vector.tensor_copy(v_sb[:, g, h, 0:DH], tr_ps)
                        else:
                            dest = {"q": q_t, "k": k_t}[tname]
                            dview = dest.rearrange("p g (s h) -> p g h s", h=H)
                            nc.vector.tensor_scalar_add(
                                out=dview[:, :, h, :],
                                in0=acc.rearrange("p (g s) -> p g s", g=NG),
                                scalar1=b_sb[:, h:h + 1],
                            )

            # duplicate q/k into partitions 64-127 for paired quadrant matmuls
            nc.gpsimd.dma_start(out=q2[DH:128, :, :], in_=q2[0:DH, :, :])
            nc.gpsimd.dma_start(out=k2[DH:128, :, :], in_=k2[0:DH, :, :])
            # head-major views: [p, g, h, s'] with s'-stride 16
            k_hm = k_t.rearrange("p g (s h) -> p g h s", h=H)
            k_hm2 = k2.rearrange("p g (s h) -> p g h s", h=H)

            # ---------- phase B: attention per group ----------
            for g in range(NG):
                # B1: S~^T -> exp -> P^T (bf16, rows t~/head-major, cols t1-order)
                ctx_ps = [
                    ps_acc.tile([DH + 1, 512], F32, tag="ctx", name=f"ctx{g}_{i}")
                    for i in range(NQG)
                ]
                for kcp in range(NKC // 2):
                    kca, kcb = 2 * kcp, 2 * kcp + 1
                    pt_a = ptp.tile([128, T], BF16, tag="pt", name=f"pt{g}_{kca}")
                    pt_b = ptp.tile([128, T], BF16, tag="pt", name=f"pt{g}_{kcb}")
                    for half in range(2):
                        ps_a = ps.tile([128, 1024], F32, tag="ps", name=f"sa{g}_{kcp}_{half}")
                        ps_b = ps.tile([128, 1024], F32, tag="ps", name=f"sb{g}_{kcp}_{half}")
                        for sub in range(2):
                            qg = half * 2 + sub
                            rhs_lo = q_t[:, g, qg * 512:(qg + 1) * 512]
                            rhs_hi = q2[DH:128, g, qg * 512:(qg + 1) * 512]
                            sl = slice(sub * 512, (sub + 1) * 512)
                            nc.tensor.matmul(
                                ps_a[0:DH, sl], lhsT=k_hm[:, g, kca, 0:DH],
                                rhs=rhs_lo, start=True, stop=True, tile_position=(0, 0),
                            )
                            nc.tensor.matmul(
                                ps_a[DH:128, sl], lhsT=k_hm2[DH:128, g, kca, DH:128],
                                rhs=rhs_hi, start=True, stop=True, tile_position=(64, 64),
                            )
                            nc.tensor.matmul(
                                ps_b[DH:128, sl], lhsT=k_hm[:, g, kcb, DH:128],
                                rhs=rhs_lo, start=True, stop=True, tile_position=(0, 64),
                            )
                            nc.tensor.matmul(
                                ps_b[0:DH, sl], lhsT=k_hm2[DH:128, g, kcb, 0:DH],
                                rhs=rhs_hi, start=True, stop=True, tile_position=(64, 0),
                            )
                        nc.scalar.activation(
                            out=pt_a[:, half * 1024:(half + 1) * 1024], in_=ps_a,
                            func=EXP, scale=SCALE,
                        )
                        nc.scalar.activation(
                            out=pt_b[:, half * 1024:(half + 1) * 1024], in_=ps_b,
                            func=EXP, scale=SCALE,
                        )
                    for qg in range(NQG):
                        nc.tensor.matmul(
                            ctx_ps[qg],
                            lhsT=v_sb[:, g, kca, :],
                            rhs=pt_a[:, qg * 512:(qg + 1) * 512],
                            start=(kca == 0), stop=False,
                        )
                        nc.tensor.matmul(
                            ctx_ps[qg],
                            lhsT=v_sb[:, g, kcb, :],
                            rhs=pt_b[:, qg * 512:(qg + 1) * 512],
                            start=False, stop=(kcb == NKC - 1),
                        )

                # evacuate unnormalized context + rowsum row, contiguous outputs
                ctx_sb = small.tile([DH + 1, T], F32, tag="ctxsb", name=f"cs{g}", bufs=2)
                for qg in range(NQG):
                    nc.vector.tensor_copy(
                        ctx_sb[:, qg * 512:(qg + 1) * 512], ctx_ps[qg]
                    )
                nc.sync.dma_start(out=ctx_out.ap()[g, :, :], in_=ctx_sb[0:DH, :])
                nc.sync.dma_start(out=rs_out.ap()[g, :], in_=ctx_sb[DH:DH + 1, :])

                # rowsum -> [128 t1, 16 chunk] via 16 K=1 matmuls (PE transpose)
                rsT_ps = ps.tile([128, NQC], F32, tag="ps", name=f"rsT{g}")
                for c16 in range(NQC):
                    nc.tensor.matmul(
                        rsT_ps[:, c16:c16 + 1],
                        lhsT=ctx_sb[DH:DH + 1, c16 * 128:(c16 + 1) * 128],
                        rhs=ones65[DH:DH + 1, :],
                        start=True, stop=True,
                    )
                recipT = small.tile([128, NQC], F32, tag="recipT", name=f"rcT{g}", bufs=4)
                nc.vector.reciprocal(out=recipT, in_=rsT_ps)
                negln = small.tile([128, NQC], F32, tag="negln", name=f"nl{g}", bufs=4)
                nc.scalar.activation(out=negln, in_=recipT, func=LN_)

                # B2: A = exp(0.5 S - ln rowsum), natural (t1, t2) order
                for qcp in range(NQC // 2):
                    qca, qcb = 2 * qcp, 2 * qcp + 1
                    a_sa = asbp.tile([128, T], F32, tag="a", name=f"a{g}_{qca}")
                    a_sbt = asbp.tile([128, T], F32, tag="a", name=f"a{g}_{qcb}")
                    for half in range(2):
                        ps_a = ps.tile([128, 1024], F32, tag="ps", name=f"ba{g}_{qcp}_{half}")
                        ps_b = ps.tile([128, 1024], F32, tag="ps", name=f"bb{g}_{qcp}_{half}")
                        for sub in range(2):
                            kg = half * 2 + sub
                            rhs_lo = k_t[:, g, kg * 512:(kg + 1) * 512]
                            rhs_hi = k2[DH:128, g, kg * 512:(kg + 1) * 512]
                            sl = slice(sub * 512, (sub + 1) * 512)
                            nc.tensor.matmul(
                                ps_a[0:DH, sl],
                                lhsT=q_t[:, g, qca * 128:qca * 128 + DH],
                                rhs=rhs_lo, start=True, stop=True, tile_position=(0, 0),
                            )
                            nc.tensor.matmul(
                                ps_a[DH:128, sl],
                                lhsT=q2[DH:128, g, qca * 128 + DH:(qca + 1) * 128],
                                rhs=rhs_hi, start=True, stop=True, tile_position=(64, 64),
                            )
                            nc.tensor.matmul(
                                ps_b[DH:128, sl],
                                lhsT=q_t[:, g, qcb * 128 + DH:(qcb + 1) * 128],
                                rhs=rhs_lo, start=True, stop=True, tile_position=(0, 64),
                            )
                            nc.tensor.matmul(
                                ps_b[0:DH, sl],
                                lhsT=q2[DH:128, g, qcb * 128:qcb * 128 + DH],
                                rhs=rhs_hi, start=True, stop=True, tile_position=(64, 0),
                            )
                        nc.scalar.activation(
                            out=a_sa[:, half * 1024:(half + 1) * 1024], in_=ps_a,
                            func=EXP, scale=SCALE, bias=negln[:, qca:qca + 1],
                        )
                        nc.scalar.activation(
                            out=a_sbt[:, half * 1024:(half + 1) * 1024], in_=ps_b,
                            func=EXP, scale=SCALE, bias=negln[:, qcb:qcb + 1],
                        )
                    nc.sync.dma_start(
                        out=attn.ap()[g, qca * 128:(qca + 1) * 128, :], in_=a_sa
                    )
                    nc.sync.dma_start(
                        out=attn.ap()[g, qcb * 128:(qcb + 1) * 128, :], in_=a_sbt
                    )

    nc.compile()
    return nc


def _build_kernel2():
    nc = bacc.Bacc("TRN2", target_bir_lowering=False, debug=False, num_devices=NCORES)
    ctxT_r = nc.dram_tensor("ctxT_r", [D, RPC], F32, kind="ExternalInput")
    scale_r = nc.dram_tensor("scale_r", [D, RPC], F32, kind="ExternalInput")
    wf = nc.dram_tensor("wf", [D, D], F32, kind="ExternalInput")
    resid = nc.dram_tensor("resid", [RPC, D], F32, kind="ExternalInput")
    ln_g = nc.dram_tensor("ln_g", [D], F32, kind="ExternalInput")
    ln_b = nc.dram_tensor("ln_b", [D], F32, kind="ExternalInput")
    normed = nc.dram_tensor("normed", [RPC, D], F32, kind="ExternalOutput")

    NRC = RPC // 128  # 4
    NOG = D // 512    # 2

    with tile.TileContext(nc) as tc:
        with (
            tc.tile_pool(name="consts", bufs=1) as consts,
            tc.tile_pool(name="work", bufs=2) as work,
            tc.tile_pool(name="stat", bufs=4) as stat,
            tc.tile_pool(name="ps", bufs=2, space="PSUM") as ps,
        ):
            ctx_raw = consts.tile([128, NK, RPC], F32)
            nc.sync.dma_start(out=ctx_raw, in_=ctxT_r.ap().rearrange("(k p) r -> p k r", p=128))
            sc_t = consts.tile([128, NK, RPC], F32)
            nc.sync.dma_start(out=sc_t, in_=scale_r.ap().rearrange("(k p) r -> p k r", p=128))
            ctx_t = consts.tile([128, NK, RPC], F32R)
            nc.vector.tensor_mul(out=ctx_t, in0=ctx_raw, in1=sc_t)
            wf_t = consts.tile([128, NK, D], F32R)
            nc.gpsimd.dma_start(out=wf_t, in_=wf.ap().rearrange("(k p) n -> p k n", p=128))
            res_t = consts.tile([128, NRC, D], F32)
            nc.gpsimd.dma_start(out=res_t, in_=resid.ap().rearrange("(rc p) n -> p rc n", p=128))
            g_bc = consts.tile([128, D], F32)
            nc.gpsimd.dma_start(out=g_bc, in_=ln_g.ap().partition_broadcast(128))
            b_bc = consts.tile([128, D], F32)
            nc.gpsimd.dma_start(out=b_bc, in_=ln_b.ap().partition_broadcast(128))
            eps_t = consts.tile([128, 1], F32)
            nc.vector.memset(eps_t, LN_EPS)

            out_view = normed.ap().rearrange("(rc p) n -> p rc n", p=128)
            for rc in range(NRC):
                x_sb = work.tile([128, D], F32, tag="x", name=f"x{rc}")
                for og in range(NOG):
                    acc = ps.tile([128, 512], F32, tag="ps", name=f"acc{rc}_{og}")
                    for k in range(NK):
                        nc.tensor.matmul(
                            acc,
                            lhsT=ctx_t[:, k, rc * 128:(rc + 1) * 128],
                            rhs=wf_t[:, k, og * 512:(og + 1) * 512],
                            start=(k == 0), stop=(k == NK - 1),
                        )
                    nc.vector.tensor_add(
                        out=x_sb[:, og * 512:(og + 1) * 512],
                        in0=acc, in1=res_t[:, rc, og * 512:(og + 1) * 512],
                    )
                stats = stat.tile([128, 2, 6], F32, tag="st", name=f"st{rc}")
                for sg in range(2):
                    nc.vector.bn_stats(
                        out=stats[:, sg, :], in_=x_sb[:, sg * 512:(sg + 1) * 512]
                    )
                mv = stat.tile([128, 2], F32, tag="mv", name=f"mv{rc}")
                nc.vector.bn_aggr(out=mv, in_=stats)
                sd = stat.tile([128, 1], F32, tag="sd", name=f"sd{rc}")
                nc.scalar.activation(out=sd, in_=mv[:, 1:2], func=SQRT, bias=eps_t)
                rstd = stat.tile([128, 1], F32, tag="rstd", name=f"rst{rc}")
                nc.vector.reciprocal(out=rstd, in_=sd)
                y_sb = work.tile([128, D], F32, tag="y", name=f"y{rc}")
                nc.vector.tensor_scalar(
                    out=y_sb, in0=x_sb, scalar1=mv[:, 0:1], scalar2=rstd,
                    op0=mybir.AluOpType.subtract, op1=mybir.AluOpType.mult,
                )
                nc.vector.tensor_mul(out=y_sb, in0=y_sb, in1=g_bc)
                nc.vector.tensor_add(out=y_sb, in0=y_sb, in1=b_bc)
                nc.gpsimd.dma_start(out=out_view[:, rc, :], in_=y_sb)

    nc.compile()
    return nc


def kernel(key, value, query, Wk, bk, Wv, bv, Wq, bq, Wf, bf, ln_g, ln_b):
    f32 = lambda a: np.ascontiguousarray(np.asarray(a, dtype=np.float32))
    key, value, query = f32(key), f32(value), f32(query)
    Wk, Wv, Wq, Wf = f32(Wk), f32(Wv), f32(Wq), f32(Wf)
    bk, bv, bq, bf, ln_g, ln_b = f32(bk), f32(bv), f32(bq), f32(bf), f32(ln_g), f32(ln_b)

    # transposed activations [D, B*S] in fp16; per-core band column slices
    qT = query.reshape(BS, D).T.astype(np.float16)
    kT = key.reshape(BS, D).T.astype(np.float16)
    vT = value.reshape(BS, D).T.astype(np.float16)
    Wq16, Wk16, Wv16 = Wq.astype(np.float16), Wk.astype(np.float16), Wv.astype(np.float16)

    def band_cols(c):
        # group order g: (b=0,m*=2c), (b=0,2c+1), (b=1,2c), (b=1,2c+1)
        cols = []
        for g in range(NG):
            b, mstar = g // BPC, 2 * c + g % BPC
            s0 = b * S + mstar * 128
            cols.append(np.arange(s0, s0 + 128))
        return np.concatenate(cols)

    if "nc1" not in _CACHE:
        _CACHE["nc1"] = _build_kernel1()
    nc1 = _CACHE["nc1"]

    bq2 = np.ascontiguousarray(bq.reshape(H, DH).T)
    bk2 = np.ascontiguousarray(bk.reshape(H, DH).T)
    bv2 = np.ascontiguousarray(bv.reshape(H, DH).T)
    in_maps1 = []
    for c in range(NCORES):
        cols = band_cols(c)
        in_maps1.append({
            "xq": np.ascontiguousarray(qT[:, cols]),
            "xk": np.ascontiguousarray(kT[:, cols]),
            "xv": np.ascontiguousarray(vT[:, cols]),
            "wq": Wq16, "wk": Wk16, "wv": Wv16,
            "bq": bq2, "bk": bk2, "bv": bv2,
        })
    res1 = run_bass_kernel_spmd(nc1, in_maps1, core_ids=list(range(NCORES)))
    _CACHE["res1"] = res1

    attention = np.empty((B * H, S, S), dtype=np.float32)
    ctxT_full = np.empty((D, BS), dtype=np.float32)
    scale_full = np.empty((D, BS), dtype=np.float32)
    for c in range(NCORES):
        r = res1.results[c]
        for g in range(NG):
            b, mstar = g // BPC, 2 * c + g % BPC
            attention[b * H + mstar] = r["attn"][g]
            fr0 = b * S + mstar * 128
            cg = r["ctx_out"][g].reshape(DH, 128, H)         # (i, s', h)
            ctxT_full[:, fr0:fr0 + 128] = (
                cg.transpose(2, 0, 1).reshape(D, 128)        # rows h*64+i
            )
            rsg = r["rs_out"][g].reshape(128, H)             # (s', h)
            scale_full[:, fr0:fr0 + 128] = np.repeat(
                (1.0 / rsg).T, DH, axis=0                    # rows h*64+i
            )

    if "nc2" not in _CACHE:
        _CACHE["nc2"] = _build_kernel2()
    nc2 = _CACHE["nc2"]

    query_flat = query.reshape(BS, D)
    in_maps2 = []
    for r in range(NCORES):
        rows = slice(r * RPC, (r + 1) * RPC)
        in_maps2.append({
            "ctxT_r": np.ascontiguousarray(ctxT_full[:, rows]),
            "scale_r": np.ascontiguousarray(scale_full[:, rows]),
            "wf": Wf,
            "resid": np.ascontiguousarray(query_flat[rows, :] + bf[None, :]),
            "ln_g": ln_g, "ln_b": ln_b,
        })
    res2 = run_bass_kernel_spmd(nc2, in_maps2, core_ids=list(range(NCORES)))
    _CACHE["res2"] = res2

    normed = np.empty((BS, D), dtype=np.float32)
    for r in range(NCORES):
        normed[r * RPC:(r + 1) * RPC, :] = res2.results[r]["normed"]
    normed = normed.reshape(B, S, D)

    return (normed, attention)
